# revision 1
# baseline (speedup 1.0000x reference)
"""Trainium2 Bass kernel for nn_ImitationHead (dense_mlp).

Computation (per batch row b of 256):
  h  = mean(z[b], spatial)                # [512] <- z [512,16,16]
  h  = relu-MLP chain 512->512->256->128->64
  goal = [goal_point[b,0,3], goal_point[b,1,3], goal_point_speed[b]]
  GRU (hidden 64, input [x(3); goal(3)]) unrolled 8 steps, each step
  followed by an output MLP 64->4(relu)->4->3 producing dx; x += dx.
  Output: the 8 x values -> [256, 8, 3].

Sharding: pure data parallel, batch 256 -> 8 cores x 32.

On-chip layout is fully "transposed" (features on partitions, batch on
the free axis) so no transposes are ever needed:
  - z shard viewed as [16384, 256]; 16 DMAs of [128p, 2, 4, 256] (1 MiB)
    where partition p holds rows {4p..4p+3} of each 512-row batch block,
    giving 4 KiB contiguous DRAM runs per partition.  The resulting
    channel permutation (chunk j, partition p <-> channel 4p+j) is
    undone by permuting the rows of the layer-1 weight on the host.
  - free-axis reduce per 256-chunk -> hT[channel_p, batch] columns,
    split 3:1 between DVE (tensor_reduce) and ACT (activation accum_out)
    so the reduction keeps pace with the DMA stream.
  - join-MLP matmuls: out_T = (W.T as lhsT).T @ h_T, weights pre-
    transposed on the host; biases applied via the ACT bias operand
    fused with the ReLU.
  - GRU: the four pre-activations (r/z gates, i_n, h_n, and the output
    MLP's first layer) live in PERSISTENT PSUM accumulators.  With
    hh' = hh - d  (d = (1-z)*(hh-n)) every W@hh term updates as
    "psum -= W@d", and the x-recurrence folds through the output MLP:
    gi_x += (W_ihx @ W23.T) @ relu(pd1).  This takes the x/output path
    off the per-step critical chain entirely.
  - biases are folded in as an extra all-ones input row (K+1 matmuls)
    at init; the 4->4 and 4->3 output layers are folded into one 4->3
    matrix on the host (no ReLU between them).
  - mean's 1/256 is folded into the first-layer weights on the host.
  - all small constants travel in one packed [128, 819] DMA.
"""

import numpy as np
from contextlib import ExitStack

N_CORES = 8
B = 256
B_SH = B // N_CORES       # 32 batch rows per core
C = 512                   # channels
S = 256                   # spatial 16*16
HID = 64
T = 8                     # pred_len
ROWS = B_SH * C           # 16384 z rows per core
N_DMA = 16                # z DMAs per core
H_PER = 2                 # batch blocks per z DMA
J = 4                     # 256-chunks per partition per batch block

# packed-constants layout: (name, partitions, cols); column offsets accumulate
_PACK = [
    ("biases", 128, 8),     # jb1 x4, jb2 x2, jb3, jb4
    ("whhbt", 65, 192),     # [W_hh.T; (0...0, b_hh_n)]           (init mms)
    ("wgobt", 4, 192),      # [W_ih[:,3:6].T; (b_rz_sum, b_ih_n)] (init mms)
    ("goalones", 4, B_SH),  # [goal.T; ones]
    ("ow1bt", 65, 4),       # [oW1.T; ob1]                        (init pd1)
    ("whhnbt", 64, 192),    # -W_hh.T                 (incremental updates)
    ("wixobt", 33, 192),    # x-path folded through d1: rows0:4 =
                            #   W23 @ W_ihx.T, row32 = W_ihx @ b23
    ("ow1nbt", 64, 4),      # -oW1.T                  (incremental pd1)
    ("ow23bt", 33, 3),      # rows0:4 = W23, row32 = b23  (output dx)
]
_OFF = {}
_ncol = 0
for _n, _p, _c in _PACK:
    _OFF[_n] = _ncol
    _ncol += _c
PACK_COLS = _ncol

_CACHE: dict = {}


def _build_program():
    import concourse.bacc as bacc
    import concourse.tile as tile
    from concourse import mybir

    f32 = mybir.dt.float32
    AF = mybir.ActivationFunctionType
    AX = mybir.AxisListType
    ALU = mybir.AluOpType

    nc = bacc.Bacc("TRN2", target_bir_lowering=False, debug=False)

    z = nc.dram_tensor("z", [ROWS, S], f32, kind="ExternalInput")
    jw1t = nc.dram_tensor("jw1t", [512, 512], f32, kind="ExternalInput")
    jw2t = nc.dram_tensor("jw2t", [512, 256], f32, kind="ExternalInput")
    jw3t = nc.dram_tensor("jw3t", [256, 128], f32, kind="ExternalInput")
    jw4t = nc.dram_tensor("jw4t", [128, 64], f32, kind="ExternalInput")
    wpack = nc.dram_tensor("wpack", [128, PACK_COLS], f32, kind="ExternalInput")
    out_d = nc.dram_tensor("out", [3 * T, B_SH], f32, kind="ExternalOutput")

    with tile.TileContext(nc) as tc, ExitStack() as ctx:
        consts = ctx.enter_context(tc.tile_pool(name="consts", bufs=1))
        zpool = ctx.enter_context(tc.tile_pool(name="zpool", bufs=3))
        hpool = ctx.enter_context(tc.tile_pool(name="hpool", bufs=1))
        work = ctx.enter_context(tc.tile_pool(name="work", bufs=2))
        xpool = ctx.enter_context(tc.tile_pool(name="xpool", bufs=2))
        psum_mlp = ctx.enter_context(
            tc.tile_pool(name="psum_mlp", bufs=2, space="PSUM"))
        psum_gru = ctx.enter_context(
            tc.tile_pool(name="psum_gru", bufs=1, space="PSUM"))

        # --- z stream: 16 x 1MiB DMAs; reduce each [128, 256] chunk ---
        # Row d*1024 + h*512 + 4p + j -> batch b = 2d+h, channel 4p+j.
        hTs = [hpool.tile([128, B_SH], f32, tag=f"hT{j}", name=f"hT{j}")
               for j in range(J)]
        junk = hpool.tile([128, S], f32)         # ACT accum main out
        z_r = z[:].rearrange("(d h p j) s -> d p h j s", h=H_PER, p=128, j=J)
        for d in range(N_DMA):
            zt = zpool.tile([128, H_PER, J, S], f32, tag="zt")
            nc.sync.dma_start(out=zt, in_=z_r[d])
            for h in range(H_PER):
                b = H_PER * d + h
                for j in range(J):
                    if j < 3:
                        nc.vector.tensor_reduce(
                            out=hTs[j][:, b:b + 1], in_=zt[:, h, j, :],
                            axis=AX.X, op=ALU.add)
                    else:
                        nc.scalar.activation(
                            out=junk, in_=zt[:, h, j, :], func=AF.Copy,
                            accum_out=hTs[j][:, b:b + 1])

        # --- constant loads: queued on the same DMA pipe AFTER the z
        # stream; ordered by when each is first needed (w1 k-chunks for
        # layer 1, then w2, then the GRU pack, then w3/w4).
        w1 = consts.tile([128, 4, 512], f32)
        jw1_r = jw1t[:].rearrange("(k p) m -> k p m", p=128)
        for k in range(4):
            nc.sync.dma_start(out=w1[:, k, :], in_=jw1_r[k])
        w2 = consts.tile([128, 4, 256], f32)
        jw2_r = jw2t[:].rearrange("(k p) m -> k p m", p=128)
        for k in range(0, 4, 2):
            nc.sync.dma_start(out=w2[:, k:k + 2, :], in_=jw2_r[k:k + 2])
        wp = consts.tile([128, PACK_COLS], f32)
        nc.sync.dma_start(out=wp, in_=wpack[:])
        w3 = consts.tile([128, 2, 128], f32)
        nc.sync.dma_start(out=w3, in_=jw3t[:].rearrange("(k p) m -> p k m", p=128))
        w4 = consts.tile([128, 64], f32)
        nc.sync.dma_start(out=w4, in_=jw4t[:])

        bs = wp[0:128, _OFF["biases"]:_OFF["biases"] + 8]
        whh = wp[0:65, _OFF["whhbt"]:_OFF["whhbt"] + 192]
        wgo = wp[0:4, _OFF["wgobt"]:_OFF["wgobt"] + 192]
        gl = wp[0:4, _OFF["goalones"]:_OFF["goalones"] + B_SH]
        ow1 = wp[0:65, _OFF["ow1bt"]:_OFF["ow1bt"] + 4]
        whhn = wp[0:64, _OFF["whhnbt"]:_OFF["whhnbt"] + 192]
        wixo = wp[0:33, _OFF["wixobt"]:_OFF["wixobt"] + 192]
        ow1n = wp[0:64, _OFF["ow1nbt"]:_OFF["ow1nbt"] + 4]
        ow23 = wp[0:33, _OFF["ow23bt"]:_OFF["ow23bt"] + 3]

        # ACT table warmup: sigmoid/tanh tables resident before the tail.
        warm = consts.tile([1, 1], f32)
        nc.vector.memset(warm, 0.0)
        nc.scalar.activation(warm, warm, AF.Sigmoid)
        nc.scalar.activation(warm, warm, AF.Tanh)

        # --- join MLP (transposed): hN_T = relu(W @ h_T + b) ---
        h1 = hpool.tile([128, 4, B_SH], f32)
        for m in range(4):
            pt = psum_mlp.tile([128, B_SH], f32, tag="mlp")
            for k in range(4):
                nc.tensor.matmul(pt, w1[:, k, m * 128:(m + 1) * 128], hTs[k],
                                 start=(k == 0), stop=(k == 3))
            nc.scalar.activation(h1[:, m, :], pt, AF.Relu, bias=bs[:, m:m + 1])
        h2 = hpool.tile([128, 2, B_SH], f32)
        for m in range(2):
            pt = psum_mlp.tile([128, B_SH], f32, tag="mlp")
            for k in range(4):
                nc.tensor.matmul(pt, w2[:, k, m * 128:(m + 1) * 128], h1[:, k, :],
                                 start=(k == 0), stop=(k == 3))
            nc.scalar.activation(h2[:, m, :], pt, AF.Relu, bias=bs[:, 4 + m:5 + m])
        h3 = hpool.tile([128, B_SH], f32)
        pt = psum_mlp.tile([128, B_SH], f32, tag="mlp")
        for k in range(2):
            nc.tensor.matmul(pt, w3[:, k, :], h2[:, k, :],
                             start=(k == 0), stop=(k == 1))
        nc.scalar.activation(h3, pt, AF.Relu, bias=bs[:, 6:7])

        # hhg rows 0:64 = GRU hidden state (in-place across steps), row 64 = 1.
        hhg = hpool.tile([65, B_SH], f32)
        nc.vector.memset(hhg[64:65, :], 1.0)
        pt = psum_mlp.tile([64, B_SH], f32, tag="mlp")
        nc.tensor.matmul(pt, w4, h3, start=True, stop=True)
        nc.scalar.activation(hhg[0:64, :], pt, AF.Relu, bias=bs[0:64, 7:8])

        # d1g: relu(pd1) with ones row at partition 32 (engine-writable);
        # rows 4:32 stay zero so the K=33 matmuls see only d1 + bias.
        d1g = hpool.tile([33, B_SH], f32)
        nc.vector.memset(d1g[0:33, :], 0.0)
        nc.vector.memset(d1g[32:33, :], 1.0)

        # --- GRU: persistent psum accumulators, 8 unrolled steps ---
        prz = psum_gru.tile([128, B_SH], f32, tag="prz")   # r/z pre-act
        pin = psum_gru.tile([64, B_SH], f32, tag="pin")    # i_n pre-act
        phn = psum_gru.tile([64, B_SH], f32, tag="phn")    # h_n pre-act
        pd1 = psum_gru.tile([4, B_SH], f32, tag="pd1")     # oW1@hh+ob1
        ptm = psum_gru.tile([64, B_SH], f32, tag="ptm")    # tanh input
        kw = dict(skip_group_check=True)
        nc.tensor.matmul(prz, wgo[:, 0:128], gl, start=True, stop=False, **kw)
        nc.tensor.matmul(prz, whh[:, 0:128], hhg, start=False, stop=False, **kw)
        nc.tensor.matmul(pin, wgo[:, 128:192], gl, start=True, stop=False, **kw)
        nc.tensor.matmul(phn, whh[:, 128:192], hhg, start=True, stop=False, **kw)
        nc.tensor.matmul(pd1, ow1[0:65, :], hhg, start=True, stop=False, **kw)

        x_prev = None
        for t in range(T):
            last = t == T - 1
            # gate path
            rz = work.tile([128, B_SH], f32, tag="rz")
            nc.scalar.activation(rz, prz, AF.Sigmoid)
            tmp = work.tile([64, B_SH], f32, tag="tmp")
            nc.vector.tensor_mul(tmp, rz[0:64, :], phn)     # r * h_n
            nc.vector.tensor_add(ptm, tmp, pin)             # + i_n -> PSUM
            zc = work.tile([64, B_SH], f32, tag="zc")
            nc.vector.tensor_scalar(
                out=zc, in0=rz[64:128, :], scalar1=-1.0, scalar2=1.0,
                op0=ALU.mult, op1=ALU.add)                  # 1 - z
            n_t = work.tile([64, B_SH], f32, tag="n_t")
            nc.scalar.activation(n_t, ptm, AF.Tanh)
            t1 = work.tile([64, B_SH], f32, tag="t1")
            nc.vector.tensor_sub(t1, hhg[0:64, :], n_t)     # hh - n
            dlt = work.tile([64, B_SH], f32, tag="dlt")
            nc.vector.tensor_mul(dlt, zc, t1)               # d = (1-z)(hh-n)

            # hh' = hh - d; pd1 first (it gates the output path), then
            # the other accumulators.
            nc.tensor.matmul(pd1, ow1n, dlt,
                             start=False, stop=last, **kw)
            nc.vector.tensor_scalar_max(d1g[0:4, :], pd1, 0.0)  # d1(hh')
            if not last:
                nc.tensor.matmul(prz, whhn[:, 0:128], dlt,
                                 start=False, stop=False, **kw)
                nc.tensor.matmul(phn, whhn[:, 128:192], dlt,
                                 start=False, stop=(t == T - 2), **kw)
                nc.vector.tensor_sub(hhg[0:64, :], hhg[0:64, :], dlt)
                # x-recurrence folded through d1g
                nc.tensor.matmul(prz, wixo[:, 0:128], d1g,
                                 start=False, stop=(t == T - 2), **kw)
                nc.tensor.matmul(pin, wixo[:, 128:192], d1g,
                                 start=False, stop=(t == T - 2), **kw)

            # x output (off the critical chain)
            pd3 = psum_gru.tile([3, B_SH], f32, tag="pd3")
            nc.tensor.matmul(pd3, ow23, d1g, start=True, stop=True)
            x_new = xpool.tile([3, B_SH], f32, tag="x")
            if x_prev is None:
                nc.vector.tensor_copy(x_new, pd3)
            else:
                nc.vector.tensor_add(x_new, x_prev, pd3)
            nc.sync.dma_start(out=out_d[3 * t:3 * t + 3, :], in_=x_new)
            x_prev = x_new

    nc.compile()
    return nc


def _get_program():
    if "nc" not in _CACHE:
        _CACHE["nc"] = _build_program()
    return _CACHE["nc"]


def make_in_maps(**inputs) -> list[dict]:
    """Host-side packing + data-parallel sharding -> one in_map per core."""
    f = lambda a: np.ascontiguousarray(np.asarray(a, dtype=np.float32))
    z = f(inputs["z"]).reshape(B, C, S)
    gp = f(inputs["goal_point"])
    gps = f(inputs["goal_point_speed"])
    W_ih, W_hh = f(inputs["W_ih"]), f(inputs["W_hh"])
    b_ih, b_hh = f(inputs["b_ih"]), f(inputs["b_hh"])
    oW1, ob1 = f(inputs["oW1"]), f(inputs["ob1"])
    oW2, ob2 = f(inputs["oW2"]), f(inputs["ob2"])
    oW3, ob3 = f(inputs["oW3"]), f(inputs["ob3"])

    # layer-1 weight: fold the 1/S mean scale and the z-layout channel
    # permutation (chunk j, partition p <-> channel 4p+j).
    jw1t = f(inputs["jW1"]).T * np.float32(1.0 / S)
    perm = (4 * np.arange(128)[None, :] + np.arange(4)[:, None]).reshape(-1)
    jw1t = np.ascontiguousarray(jw1t[perm])
    jw2t = np.ascontiguousarray(f(inputs["jW2"]).T)
    jw3t = np.ascontiguousarray(f(inputs["jW3"]).T)
    jw4t = np.ascontiguousarray(f(inputs["jW4"]).T)

    # bias pack [128, 8]: jb1 (4 cols), jb2 (2), jb3 (1), jb4 (1, rows 0:64)
    biases = np.zeros((128, 8), np.float32)
    biases[:, 0:4] = f(inputs["jb1"]).reshape(4, 128).T
    biases[:, 4:6] = f(inputs["jb2"]).reshape(2, 128).T
    biases[:, 6] = f(inputs["jb3"])
    biases[0:64, 7] = f(inputs["jb4"])

    brow = np.concatenate([b_ih[0:128] + b_hh[0:128], b_ih[128:192]])
    wgobt = np.concatenate([W_ih[:, 3:6].T, brow[None, :]])  # [4, 192]
    brow2 = np.concatenate([np.zeros(128, np.float32), b_hh[128:192]])
    whhbt = np.concatenate([W_hh.T, brow2[None, :]])         # [65, 192]
    whhnbt = -W_hh.T                                         # [64, 192]

    ow1bt = np.concatenate([oW1.T, ob1[None, :]])            # [65, 4]
    ow1nbt = -oW1.T                                          # [64, 4]
    w23 = oW2.T @ oW3.T                                      # [4, 3]
    b23 = ob2 @ oW3.T + ob3                                  # [3]
    ow23bt = np.zeros((33, 3), np.float32)
    ow23bt[0:4] = w23
    ow23bt[32] = b23
    # x-recurrence folded through d1:  W_ihx @ dx = (W23 @ W_ihx.T).T@d1...
    wixobt = np.zeros((33, 192), np.float32)
    wixobt[0:4] = w23 @ W_ih[:, 0:3].T                       # [4, 192]
    wixobt[32] = W_ih[:, 0:3] @ b23                          # [192]

    goalT = np.stack([gp[:, 0, 3], gp[:, 1, 3], gps])        # [3, 256]

    segs = dict(biases=biases, whhbt=whhbt, wgobt=wgobt, ow1bt=ow1bt,
                whhnbt=whhnbt, wixobt=wixobt, ow1nbt=ow1nbt, ow23bt=ow23bt)
    in_maps = []
    for i in range(N_CORES):
        sl = slice(i * B_SH, (i + 1) * B_SH)
        go = np.concatenate(
            [goalT[:, sl], np.ones((1, B_SH), np.float32)])  # [4, 32]
        pack = np.zeros((128, PACK_COLS), np.float32)
        for name, parts, cols in _PACK:
            arr = go if name == "goalones" else segs[name]
            pack[0:parts, _OFF[name]:_OFF[name] + cols] = arr
        in_maps.append(dict(
            z=np.ascontiguousarray(z[sl].reshape(ROWS, S)),
            jw1t=jw1t, jw2t=jw2t, jw3t=jw3t, jw4t=jw4t,
            wpack=pack,
        ))
    return in_maps


def unshard_out(results: list[dict]) -> np.ndarray:
    # per-core out [24, 32]: row 3t+c, col b  ->  [32, 8, 3]
    parts = [r["out"].reshape(T, 3, B_SH).transpose(2, 0, 1) for r in results]
    return np.ascontiguousarray(np.concatenate(parts, axis=0), dtype=np.float32)


def kernel(**inputs) -> np.ndarray:
    from concourse.bass_utils import run_bass_kernel_spmd

    nc = _get_program()
    in_maps = make_in_maps(**inputs)
    res = run_bass_kernel_spmd(nc, in_maps, core_ids=list(range(N_CORES)))
    return unshard_out(res.results)



# revision 7
# speedup vs baseline: 1.1061x; 1.1061x over previous
"""Trainium2 Bass kernel for nn_ImitationHead (dense_mlp).

Computation (per batch row b of 256):
  h  = mean(z[b], spatial)                # [512] <- z [512,16,16]
  h  = relu-MLP chain 512->512->256->128->64
  goal = [goal_point[b,0,3], goal_point[b,1,3], goal_point_speed[b]]
  GRU (hidden 64, input [x(3); goal(3)]) unrolled 8 steps, each step
  followed by an output MLP 64->4(relu)->4->3 producing dx; x += dx.
  Output: the 8 x values -> [256, 8, 3].

Sharding: pure data parallel, batch 256 -> 8 cores x 32.

Key layout/perf choices (v2):
  - z and all join-MLP weights travel as float16: halves the HBM
    traffic that dominates the kernel (8 MiB z + 0.9 MiB weights per
    core).  fp16 keeps 10 mantissa bits so the 2e-2 tolerance is safe.
  - on-chip layout fully "transposed" (features on partitions, batch on
    the free axis); z shard viewed as [16384, 256] f16, 8 DMAs of
    [128p, 4, 4, 256] (1 MiB) with 2 KiB contiguous DRAM runs.  The
    channel permutation (chunk j, partition p <-> channel 4p+j) is
    undone by permuting the rows of the layer-1 weight on the host.
  - spatial-sum reduces are split DVE / Pool (j-outer so the first L1
    k-chunk can start right after the last DMA's first reduce group).
  - join MLP matmuls in fp16 (1 PE cycle/row instead of 4); bias+ReLU
    fused on the Pool engine via tensor_scalar with a per-partition
    bias AP (no ACT round-trip).
  - GRU: persistent PSUM accumulators; hh' = hh - d with
    d = (1-z)*(hh-n); the x-recurrence folds through the output MLP.
    All elementwise ops on Pool, sigmoid/tanh on ACT writing PSUM,
    per-step incremental matmuls in fp16.  Biases fold in as an extra
    all-ones input row at init; output 4->4 and 4->3 layers fold into
    one 4->3 matrix on the host; mean's 1/256 folds into W1.
"""

import numpy as np
from contextlib import ExitStack

N_CORES = 8
B = 256
B_SH = B // N_CORES       # 32 batch rows per core
C = 512                   # channels
S = 256                   # spatial 16*16
HID = 64
T = 8                     # pred_len
ROWS = B_SH * C           # 16384 z rows per core
N_DMA = 8                 # z DMAs per core (1 MiB f16 each)
H_PER = 4                 # batch blocks per z DMA
J = 4                     # 256-chunks per partition per batch block

# f32 constant pack [65, 420]
_OFF32 = dict(whhbt=0, wgobt=192, goalones=384, ow1bt=416)
P32_COLS = 420
# f16 constant pack [64, 391]
_OFF16 = dict(whhnbt=0, wixobt=192, ow1nbt=384, ow23bt=388)
P16_COLS = 391

_CACHE: dict = {}


def _build_program():
    import concourse.bacc as bacc
    import concourse.tile as tile
    from concourse import mybir

    f32 = mybir.dt.float32
    f16 = mybir.dt.float16
    AF = mybir.ActivationFunctionType
    AX = mybir.AxisListType
    ALU = mybir.AluOpType

    nc = bacc.Bacc("TRN2", target_bir_lowering=False, debug=False)

    z = nc.dram_tensor("z", [ROWS, S], f16, kind="ExternalInput")
    w1d = nc.dram_tensor("w1", [512, 512], f16, kind="ExternalInput")
    w2d = nc.dram_tensor("w2", [512, 256], f16, kind="ExternalInput")
    w34d = nc.dram_tensor("w34", [128, 320], f16, kind="ExternalInput")
    wbias_d = nc.dram_tensor("wbias", [128, 8], f32, kind="ExternalInput")
    wp32_d = nc.dram_tensor("wp32", [65, P32_COLS], f32, kind="ExternalInput")
    wp16_d = nc.dram_tensor("wp16", [64, P16_COLS], f16, kind="ExternalInput")
    out_d = nc.dram_tensor("out", [3 * T, B_SH], f32, kind="ExternalOutput")

    with tile.TileContext(nc) as tc, ExitStack() as ctx, \
            nc.allow_low_precision(reason="fp16 pipeline; output tol 2e-2"):
        consts = ctx.enter_context(tc.tile_pool(name="consts", bufs=1))
        zpool = ctx.enter_context(tc.tile_pool(name="zpool", bufs=3))
        hpool = ctx.enter_context(tc.tile_pool(name="hpool", bufs=1))
        work = ctx.enter_context(tc.tile_pool(name="work", bufs=2))
        xpool = ctx.enter_context(tc.tile_pool(name="xpool", bufs=2))
        psum_mlp = ctx.enter_context(
            tc.tile_pool(name="psum_mlp", bufs=2, space="PSUM"))
        psum_gru = ctx.enter_context(
            tc.tile_pool(name="psum_gru", bufs=1, space="PSUM"))

        # --- small constant loads first (init matmuls depend on them) ---
        wb = consts.tile([128, 8], f32)
        nc.sync.dma_start(out=wb, in_=wbias_d[:])
        wp32 = consts.tile([65, P32_COLS], f32)
        nc.sync.dma_start(out=wp32, in_=wp32_d[:])
        wp16 = consts.tile([64, P16_COLS], f16)
        nc.sync.dma_start(out=wp16, in_=wp16_d[:])

        whh = wp32[0:65, 0:192]
        wgo = wp32[0:4, 192:384]
        gl = wp32[0:4, 384:384 + B_SH]
        ow1 = wp32[0:65, 416:420]
        whhn = wp16[0:64, 0:192]
        wixo = wp16[0:33, 192:384]
        ow1n = wp16[0:64, 384:388]
        ow23 = wp16[0:33, 388:391]

        # ACT table warmup: sigmoid/tanh tables resident before the tail.
        warm = consts.tile([1, 1], f32)
        nc.vector.memset(warm, 0.0)
        nc.scalar.activation(warm, warm, AF.Sigmoid)
        nc.scalar.activation(warm, warm, AF.Tanh)

        # hhg rows 0:64 = GRU hidden state (in-place across steps), row 64 = 1.
        hhg = hpool.tile([65, B_SH], f32)
        nc.vector.memset(hhg[64:65, :], 1.0)
        # d1g: relu(pd1) with ones row at partition 32; rows 4:32 stay zero
        # so the K=33 matmuls see only d1 + bias.
        d1g = hpool.tile([33, B_SH], f16)
        nc.vector.memset(d1g[0:33, :], 0.0)
        nc.vector.memset(d1g[32:33, :], 1.0)

        # GRU goal-path init matmuls: depend only on wp32, run during z.
        kw = dict(skip_group_check=True)
        prz = psum_gru.tile([128, B_SH], f32, tag="prz")   # r/z pre-act
        pin = psum_gru.tile([64, B_SH], f32, tag="pin")    # i_n pre-act
        phn = psum_gru.tile([64, B_SH], f32, tag="phn")    # h_n pre-act
        pd1 = psum_gru.tile([4, B_SH], f32, tag="pd1")     # oW1@hh+ob1
        nc.tensor.matmul(prz, wgo[:, 0:128], gl, start=True, stop=False, **kw)
        nc.tensor.matmul(pin, wgo[:, 128:192], gl, start=True, stop=False, **kw)

        # --- z stream: 8 x 1MiB f16 DMAs; one multi-axis DVE reduce per
        # batch row: [128, J, 256] -> [128, J] columns of hTc.
        # Row d*2048 + h*512 + 4p + j -> batch b = 4d+h, channel 4p+j.
        hTc = hpool.tile([128, B_SH, J], f16)
        z_r = z[:].rearrange("(d h p j) s -> d p h j s", h=H_PER, p=128, j=J)
        for d in range(N_DMA):
            zt = zpool.tile([128, H_PER, J, S], f16, tag="zt")
            nc.sync.dma_start(out=zt, in_=z_r[d])
            for h in range(H_PER):
                b = H_PER * d + h
                nc.vector.tensor_reduce(
                    out=hTc[:, b:b + 1, :], in_=zt[:, h, :, :],
                    axis=AX.X, op=ALU.add)

        # --- MLP weights queued after the z stream (needed later) ---
        w1 = consts.tile([128, 4, 512], f16)
        jw1_r = w1d[:].rearrange("(k p) m -> k p m", p=128)
        for k in range(0, 4, 2):
            nc.sync.dma_start(out=w1[:, k:k + 2, :], in_=jw1_r[k:k + 2])
        w2 = consts.tile([128, 4, 256], f16)
        nc.sync.dma_start(out=w2, in_=w2d[:].rearrange("(k p) m -> p k m", p=128))
        w34 = consts.tile([128, 320], f16)
        nc.sync.dma_start(out=w34, in_=w34d[:])

        # --- join MLP (transposed): hN_T = relu(W @ h_T + b) ---
        # bias+relu fused on Pool: (psum + bias) max 0 -> f16
        h1 = hpool.tile([128, 4, B_SH], f16)
        for m in range(4):
            pt = psum_mlp.tile([128, B_SH], f32, tag="mlp")
            for k in range(4):
                nc.tensor.matmul(pt, w1[:, k, m * 128:(m + 1) * 128],
                                 hTc[:, :, k],
                                 start=(k == 0), stop=(k == 3))
            nc.vector.tensor_scalar(
                out=h1[:, m, :], in0=pt, scalar1=wb[:, m:m + 1], scalar2=0.0,
                op0=ALU.add, op1=ALU.max)
        h2 = hpool.tile([128, 2, B_SH], f16)
        for m in range(2):
            pt = psum_mlp.tile([128, B_SH], f32, tag="mlp")
            for k in range(4):
                nc.tensor.matmul(pt, w2[:, k, m * 128:(m + 1) * 128], h1[:, k, :],
                                 start=(k == 0), stop=(k == 3))
            nc.vector.tensor_scalar(
                out=h2[:, m, :], in0=pt, scalar1=wb[:, 4 + m:5 + m], scalar2=0.0,
                op0=ALU.add, op1=ALU.max)
        h3 = hpool.tile([128, B_SH], f16)
        pt = psum_mlp.tile([128, B_SH], f32, tag="mlp")
        for k in range(2):
            nc.tensor.matmul(pt, w34[:, k * 128:(k + 1) * 128], h2[:, k, :],
                             start=(k == 0), stop=(k == 1))
        nc.vector.tensor_scalar(
            out=h3, in0=pt, scalar1=wb[:, 6:7], scalar2=0.0,
            op0=ALU.add, op1=ALU.max)
        pt = psum_mlp.tile([64, B_SH], f32, tag="mlp")
        nc.tensor.matmul(pt, w34[:, 256:320], h3, start=True, stop=True)
        nc.vector.tensor_scalar(
            out=hhg[0:64, :], in0=pt, scalar1=wb[0:64, 7:8], scalar2=0.0,
            op0=ALU.add, op1=ALU.max)

        # GRU hidden-path init matmuls (f32 operands, one-time).
        nc.tensor.matmul(prz, whh[:, 0:128], hhg, start=False, stop=False, **kw)
        nc.tensor.matmul(phn, whh[:, 128:192], hhg, start=True, stop=False, **kw)
        nc.tensor.matmul(pd1, ow1, hhg, start=True, stop=False, **kw)
        phn_s = work.tile([64, B_SH], f32, tag="phn_s")
        nc.vector.tensor_copy(phn_s, phn)
        pin_s = work.tile([64, B_SH], f32, tag="pin_s")
        nc.vector.tensor_copy(pin_s, pin)

        # --- GRU: persistent psum accumulators, 8 unrolled steps.
        # Pool (SBUF-only) runs the elementwise chain; DVE shadows pin/phn
        # into SBUF during the sigmoid and handles the PSUM-touching relu
        # and x-add; ACT does sigmoid/tanh.
        x_prev = None
        for t in range(T):
            last = t == T - 1
            rz = work.tile([128, B_SH], f32, tag="rz")
            nc.scalar.activation(rz, prz, AF.Sigmoid)
            tmp = work.tile([64, B_SH], f32, tag="tmp")
            nc.gpsimd.tensor_mul(tmp, rz[0:64, :], phn_s)   # r * h_n
            ptm = work.tile([64, B_SH], f32, tag="ptm")
            nc.gpsimd.tensor_add(ptm, tmp, pin_s)           # + i_n
            zc = work.tile([64, B_SH], f32, tag="zc")
            nc.gpsimd.tensor_scalar(
                out=zc, in0=rz[64:128, :], scalar1=-1.0, scalar2=1.0,
                op0=ALU.mult, op1=ALU.add)                  # 1 - z
            n_t = work.tile([64, B_SH], f32, tag="n_t")
            nc.scalar.activation(n_t, ptm, AF.Tanh)
            t1 = work.tile([64, B_SH], f32, tag="t1")
            nc.gpsimd.tensor_sub(t1, hhg[0:64, :], n_t)     # hh - n
            dlt = work.tile([64, B_SH], f16, tag="dlt")
            nc.gpsimd.tensor_mul(dlt, zc, t1)               # d = (1-z)(hh-n)

            # hh' = hh - d; pd1 first (it gates the output path); the
            # whhn updates are ready before the relu, so PE runs them
            # during the relu's sem latency.
            nc.tensor.matmul(pd1, ow1n, dlt,
                             start=False, stop=last, **kw)
            if not last:
                nc.tensor.matmul(prz, whhn[:, 0:128], dlt,
                                 start=False, stop=False, **kw)
                nc.tensor.matmul(phn, whhn[:, 128:192], dlt,
                                 start=False, stop=(t == T - 2), **kw)
                dlt32 = work.tile([64, B_SH], f32, tag="dlt32")
                nc.gpsimd.tensor_mul(dlt32, zc, t1)
                nc.gpsimd.tensor_sub(hhg[0:64, :], hhg[0:64, :], dlt32)
            nc.vector.tensor_scalar_max(d1g[0:4, :], pd1, 0.0)  # d1(hh')
            if not last:
                # x-recurrence folded through d1g
                nc.tensor.matmul(prz, wixo[:, 0:128], d1g,
                                 start=False, stop=(t == T - 2), **kw)
                nc.tensor.matmul(pin, wixo[:, 128:192], d1g,
                                 start=False, stop=(t == T - 2), **kw)

            # x output (off the critical chain)
            pd3 = psum_gru.tile([3, B_SH], f32, tag="pd3")
            nc.tensor.matmul(pd3, ow23, d1g, start=True, stop=True)
            if not last:
                # shadow next step's h_n / i_n pre-acts into SBUF (DVE,
                # overlapped with the next sigmoid)
                phn_s = work.tile([64, B_SH], f32, tag="phn_s")
                nc.vector.tensor_copy(phn_s, phn)
                pin_s = work.tile([64, B_SH], f32, tag="pin_s")
                nc.vector.tensor_copy(pin_s, pin)
            x_new = xpool.tile([3, B_SH], f32, tag="x")
            if x_prev is None:
                nc.vector.tensor_copy(x_new, pd3)
            else:
                nc.vector.tensor_add(x_new, x_prev, pd3)
            nc.sync.dma_start(out=out_d[3 * t:3 * t + 3, :], in_=x_new)
            x_prev = x_new

    nc.compile()
    return nc


def _get_program():
    if "nc" not in _CACHE:
        _CACHE["nc"] = _build_program()
    return _CACHE["nc"]


def make_in_maps(**inputs) -> list[dict]:
    """Host-side packing + data-parallel sharding -> one in_map per core."""
    f = lambda a: np.ascontiguousarray(np.asarray(a, dtype=np.float32))
    z = f(inputs["z"]).reshape(B, C, S)
    gp = f(inputs["goal_point"])
    gps = f(inputs["goal_point_speed"])
    W_ih, W_hh = f(inputs["W_ih"]), f(inputs["W_hh"])
    b_ih, b_hh = f(inputs["b_ih"]), f(inputs["b_hh"])
    oW1, ob1 = f(inputs["oW1"]), f(inputs["ob1"])
    oW2, ob2 = f(inputs["oW2"]), f(inputs["ob2"])
    oW3, ob3 = f(inputs["oW3"]), f(inputs["ob3"])

    # layer-1 weight: fold the 1/S mean scale and the z-layout channel
    # permutation (chunk j, partition p <-> channel 4p+j).
    jw1t = f(inputs["jW1"]).T * np.float32(1.0 / S)
    perm = (4 * np.arange(128)[None, :] + np.arange(4)[:, None]).reshape(-1)
    w1 = np.ascontiguousarray(jw1t[perm]).astype(np.float16)
    w2 = np.ascontiguousarray(f(inputs["jW2"]).T).astype(np.float16)
    jw3t = f(inputs["jW3"]).T.astype(np.float16)                 # [256, 128]
    jw4t = f(inputs["jW4"]).T.astype(np.float16)                 # [128, 64]
    w34 = np.zeros((128, 320), np.float16)
    w34[:, 0:128] = jw3t[0:128]
    w34[:, 128:256] = jw3t[128:256]
    w34[:, 256:320] = jw4t

    # bias pack [128, 8]: jb1 (4 cols), jb2 (2), jb3 (1), jb4 (1, rows 0:64)
    wbias = np.zeros((128, 8), np.float32)
    wbias[:, 0:4] = f(inputs["jb1"]).reshape(4, 128).T
    wbias[:, 4:6] = f(inputs["jb2"]).reshape(2, 128).T
    wbias[:, 6] = f(inputs["jb3"])
    wbias[0:64, 7] = f(inputs["jb4"])

    brow = np.concatenate([b_ih[0:128] + b_hh[0:128], b_ih[128:192]])
    wgobt = np.concatenate([W_ih[:, 3:6].T, brow[None, :]])  # [4, 192]
    brow2 = np.concatenate([np.zeros(128, np.float32), b_hh[128:192]])
    whhbt = np.concatenate([W_hh.T, brow2[None, :]])         # [65, 192]
    ow1bt = np.concatenate([oW1.T, ob1[None, :]])            # [65, 4]

    w23 = oW2.T @ oW3.T                                      # [4, 3]
    b23 = ob2 @ oW3.T + ob3                                  # [3]
    wp16 = np.zeros((64, P16_COLS), np.float16)
    wp16[0:64, 0:192] = -W_hh.T
    wp16[0:4, 192:384] = w23 @ W_ih[:, 0:3].T
    wp16[32, 192:384] = W_ih[:, 0:3] @ b23
    wp16[0:64, 384:388] = -oW1.T
    wp16[0:4, 388:391] = w23
    wp16[32, 388:391] = b23

    goalT = np.stack([gp[:, 0, 3], gp[:, 1, 3], gps])        # [3, 256]

    z16 = z.astype(np.float16)

    in_maps = []
    for i in range(N_CORES):
        sl = slice(i * B_SH, (i + 1) * B_SH)
        wp32 = np.zeros((65, P32_COLS), np.float32)
        wp32[0:65, 0:192] = whhbt
        wp32[0:4, 192:384] = wgobt
        wp32[0:3, 384:384 + B_SH] = goalT[:, sl]
        wp32[3, 384:384 + B_SH] = 1.0
        wp32[0:65, 416:420] = ow1bt
        in_maps.append(dict(
            z=np.ascontiguousarray(z16[sl].reshape(ROWS, S)),
            w1=w1, w2=w2, w34=w34, wbias=wbias,
            wp32=wp32, wp16=wp16,
        ))
    return in_maps


def unshard_out(results: list[dict]) -> np.ndarray:
    # per-core out [24, 32]: row 3t+c, col b  ->  [32, 8, 3]
    parts = [r["out"].reshape(T, 3, B_SH).transpose(2, 0, 1) for r in results]
    return np.ascontiguousarray(np.concatenate(parts, axis=0), dtype=np.float32)


def kernel(**inputs) -> np.ndarray:
    from concourse.bass_utils import run_bass_kernel_spmd

    nc = _get_program()
    in_maps = make_in_maps(**inputs)
    res = run_bass_kernel_spmd(nc, in_maps, core_ids=list(range(N_CORES)))
    return unshard_out(res.results)


# revision 9
# speedup vs baseline: 1.2394x; 1.1205x over previous
"""Trainium2 Bass kernel for nn_ImitationHead (dense_mlp).

Computation (per batch row b of 256):
  h  = mean(z[b], spatial)                # [512] <- z [512,16,16]
  h  = relu-MLP chain 512->512->256->128->64
  goal = [goal_point[b,0,3], goal_point[b,1,3], goal_point_speed[b]]
  GRU (hidden 64, input [x(3); goal(3)]) unrolled 8 steps, each step
  followed by an output MLP 64->4(relu)->4->3 producing dx; x += dx.
  Output: the 8 x values -> [256, 8, 3].

Sharding: pure data parallel, batch 256 -> 8 cores x 32.

Key layout/perf choices (v2):
  - z and all join-MLP weights travel as float16: halves the HBM
    traffic that dominates the kernel (8 MiB z + 0.9 MiB weights per
    core).  fp16 keeps 10 mantissa bits so the 2e-2 tolerance is safe.
  - on-chip layout fully "transposed" (features on partitions, batch on
    the free axis); z shard viewed as [16384, 256] f16, 8 DMAs of
    [128p, 4, 4, 256] (1 MiB) with 2 KiB contiguous DRAM runs.  The
    channel permutation (chunk j, partition p <-> channel 4p+j) is
    undone by permuting the rows of the layer-1 weight on the host.
  - spatial-sum reduces are split DVE / Pool (j-outer so the first L1
    k-chunk can start right after the last DMA's first reduce group).
  - join MLP matmuls in fp16 (1 PE cycle/row instead of 4); bias+ReLU
    fused on the Pool engine via tensor_scalar with a per-partition
    bias AP (no ACT round-trip).
  - GRU: persistent PSUM accumulators; hh' = hh - d with
    d = (1-z)*(hh-n); the x-recurrence folds through the output MLP.
    All elementwise ops on Pool, sigmoid/tanh on ACT writing PSUM,
    per-step incremental matmuls in fp16.  Biases fold in as an extra
    all-ones input row at init; output 4->4 and 4->3 layers fold into
    one 4->3 matrix on the host; mean's 1/256 folds into W1.
"""

import numpy as np
from contextlib import ExitStack

N_CORES = 8
B = 256
B_SH = B // N_CORES       # 32 batch rows per core
C = 512                   # channels
S = 256                   # spatial 16*16
HID = 64
T = 8                     # pred_len
ROWS = B_SH * C           # 16384 z rows per core
N_DMA = 8                 # z DMAs per core (1 MiB f16 each)
H_PER = 4                 # batch blocks per z DMA
J = 4                     # 256-chunks per partition per batch block

# f32 constant pack [65, 420]
_OFF32 = dict(whhbt=0, wgobt=192, goalones=384, ow1bt=416)
P32_COLS = 420
# f16 constant pack [64, 391]
_OFF16 = dict(whhnbt=0, wixobt=192, ow1nbt=384, ow23bt=388)
P16_COLS = 391

_CACHE: dict = {}


def _build_program():
    import concourse.bacc as bacc
    import concourse.tile as tile
    from concourse import mybir

    f32 = mybir.dt.float32
    f16 = mybir.dt.float16
    AF = mybir.ActivationFunctionType
    AX = mybir.AxisListType
    ALU = mybir.AluOpType

    nc = bacc.Bacc("TRN2", target_bir_lowering=False, debug=False)

    z = nc.dram_tensor("z", [ROWS, S], f16, kind="ExternalInput")
    w1d = nc.dram_tensor("w1", [512, 512], f16, kind="ExternalInput")
    w2d = nc.dram_tensor("w2", [512, 256], f16, kind="ExternalInput")
    w34d = nc.dram_tensor("w34", [128, 320], f16, kind="ExternalInput")
    wbias_d = nc.dram_tensor("wbias", [128, 8], f32, kind="ExternalInput")
    wp32_d = nc.dram_tensor("wp32", [65, P32_COLS], f32, kind="ExternalInput")
    wp16_d = nc.dram_tensor("wp16", [64, P16_COLS], f16, kind="ExternalInput")
    out_d = nc.dram_tensor("out", [3 * T, B_SH], f32, kind="ExternalOutput")

    with tile.TileContext(nc) as tc, ExitStack() as ctx, \
            nc.allow_low_precision(reason="fp16 pipeline; output tol 2e-2"):
        consts = ctx.enter_context(tc.tile_pool(name="consts", bufs=1))
        zpool = ctx.enter_context(tc.tile_pool(name="zpool", bufs=3))
        hpool = ctx.enter_context(tc.tile_pool(name="hpool", bufs=1))
        work = ctx.enter_context(tc.tile_pool(name="work", bufs=2))
        xpool = ctx.enter_context(tc.tile_pool(name="xpool", bufs=2))
        psum_mlp = ctx.enter_context(
            tc.tile_pool(name="psum_mlp", bufs=2, space="PSUM"))
        psum_gru = ctx.enter_context(
            tc.tile_pool(name="psum_gru", bufs=1, space="PSUM"))

        # --- small constant loads first (init matmuls depend on them) ---
        wb = consts.tile([128, 8], f32)
        nc.sync.dma_start(out=wb, in_=wbias_d[:])
        wp32 = consts.tile([65, P32_COLS], f32)
        nc.sync.dma_start(out=wp32, in_=wp32_d[:])
        wp16 = consts.tile([64, P16_COLS], f16)
        nc.sync.dma_start(out=wp16, in_=wp16_d[:])

        whh = wp32[0:65, 0:192]
        wgo = wp32[0:4, 192:384]
        gl = wp32[0:4, 384:384 + B_SH]
        ow1 = wp32[0:65, 416:420]
        whhn = wp16[0:64, 0:192]
        wixo = wp16[0:33, 192:384]
        ow1n = wp16[0:64, 384:388]
        ow23 = wp16[0:33, 388:391]

        # ACT table warmup: sigmoid/tanh tables resident before the tail.
        warm = consts.tile([1, 1], f32)
        nc.vector.memset(warm, 0.0)
        nc.scalar.activation(warm, warm, AF.Sigmoid)
        nc.scalar.activation(warm, warm, AF.Tanh)

        # hhg rows 0:64 = GRU hidden state (in-place across steps), row 64 = 1.
        hhg = hpool.tile([65, B_SH], f32)
        nc.vector.memset(hhg[64:65, :], 1.0)
        # d1g: relu(pd1) with ones row at partition 32; rows 4:32 stay zero
        # so the K=33 matmuls see only d1 + bias.
        d1g = hpool.tile([33, B_SH], f16)
        nc.vector.memset(d1g[0:33, :], 0.0)
        nc.vector.memset(d1g[32:33, :], 1.0)

        # GRU goal-path init matmuls: depend only on wp32, run during z.
        kw = dict(skip_group_check=True)
        prz = psum_gru.tile([128, B_SH], f32, tag="prz")   # r/z pre-act
        pin = psum_gru.tile([64, B_SH], f32, tag="pin")    # i_n pre-act
        phn = psum_gru.tile([64, B_SH], f32, tag="phn")    # h_n pre-act
        pd1 = psum_gru.tile([4, B_SH], f32, tag="pd1")     # oW1@hh+ob1
        nc.tensor.matmul(prz, wgo[:, 0:128], gl, start=True, stop=False, **kw)
        nc.tensor.matmul(pin, wgo[:, 128:192], gl, start=True, stop=False, **kw)

        # --- z stream: 8 x 1MiB f16 DMAs.  TensorReduce runs at 1x on
        # DVE but TensorTensor fp16 gets the 2x perf mode, so the spatial
        # sum is a binary tree of fp16 adds (2.6us per 1MiB DMA < 2.9us
        # DMA time).  The last DMA is split into 4 quarter DMAs so the
        # final tree only trails the stream by ~1us.
        # Row d*2048 + h*512 + 4p + j -> batch b = 4d+h, channel 4p+j.
        hTc = hpool.tile([128, B_SH, J], f16)
        z_r = z[:].rearrange("(d h p j) s -> d p h j s", h=H_PER, p=128, j=J)

        def tree_reduce(src_ap, n_rows, out_ap):
            # src_ap: [128, n_rows, S] fp16; sums S columns -> out_ap
            # [128, n_rows(, J... caller-shaped)] via ping-pong adds.
            scA = work.tile([128, n_rows, S // 2], f16, tag=f"trA{n_rows}")
            scB = work.tile([128, n_rows, S // 4], f16, tag=f"trB{n_rows}")
            n = S // 2
            nc.vector.tensor_add(scA[:, :, 0:n], src_ap[:, :, 0:n],
                                 src_ap[:, :, n:2 * n])
            cur = scA
            oth = scB
            while n > 2:
                h_n = n // 2
                nc.vector.tensor_add(oth[:, :, 0:h_n], cur[:, :, 0:h_n],
                                     cur[:, :, h_n:n])
                cur, oth = oth, cur
                n = h_n
            nc.vector.tensor_add(out_ap, cur[:, :, 0:1], cur[:, :, 1:2])

        for d in range(N_DMA - 1):
            zt = zpool.tile([128, H_PER, J, S], f16, tag="zt")
            nc.sync.dma_start(out=zt, in_=z_r[d])
            b = H_PER * d
            tree_reduce(zt[:].rearrange("p h j s -> p (h j) s"), H_PER * J,
                        hTc[:, b:b + H_PER, :].rearrange("p b j -> p (b j) ()"))
        d = N_DMA - 1
        for h in range(H_PER):
            zq = zpool.tile([128, 1, J, S], f16, tag="zq")
            nc.sync.dma_start(out=zq, in_=z_r[d][:, h:h + 1])
            b = H_PER * d + h
            tree_reduce(zq[:].rearrange("p h j s -> p (h j) s"), J,
                        hTc[:, b:b + 1, :].rearrange("p b j -> p (b j) ()"))

        # --- MLP weights queued after the z stream (needed later) ---
        w1 = consts.tile([128, 4, 512], f16)
        jw1_r = w1d[:].rearrange("(k p) m -> k p m", p=128)
        for k in range(0, 4, 2):
            nc.sync.dma_start(out=w1[:, k:k + 2, :], in_=jw1_r[k:k + 2])
        w2 = consts.tile([128, 4, 256], f16)
        nc.sync.dma_start(out=w2, in_=w2d[:].rearrange("(k p) m -> p k m", p=128))
        w34 = consts.tile([128, 320], f16)
        nc.sync.dma_start(out=w34, in_=w34d[:])

        # --- join MLP (transposed): hN_T = relu(W @ h_T + b) ---
        # bias+relu fused on Pool: (psum + bias) max 0 -> f16
        h1 = hpool.tile([128, 4, B_SH], f16)
        for m in range(4):
            pt = psum_mlp.tile([128, B_SH], f32, tag="mlp")
            for k in range(4):
                nc.tensor.matmul(pt, w1[:, k, m * 128:(m + 1) * 128],
                                 hTc[:, :, k],
                                 start=(k == 0), stop=(k == 3))
            nc.vector.tensor_scalar(
                out=h1[:, m, :], in0=pt, scalar1=wb[:, m:m + 1], scalar2=0.0,
                op0=ALU.add, op1=ALU.max)
        h2 = hpool.tile([128, 2, B_SH], f16)
        for m in range(2):
            pt = psum_mlp.tile([128, B_SH], f32, tag="mlp")
            for k in range(4):
                nc.tensor.matmul(pt, w2[:, k, m * 128:(m + 1) * 128], h1[:, k, :],
                                 start=(k == 0), stop=(k == 3))
            nc.vector.tensor_scalar(
                out=h2[:, m, :], in0=pt, scalar1=wb[:, 4 + m:5 + m], scalar2=0.0,
                op0=ALU.add, op1=ALU.max)
        h3 = hpool.tile([128, B_SH], f16)
        pt = psum_mlp.tile([128, B_SH], f32, tag="mlp")
        for k in range(2):
            nc.tensor.matmul(pt, w34[:, k * 128:(k + 1) * 128], h2[:, k, :],
                             start=(k == 0), stop=(k == 1))
        nc.vector.tensor_scalar(
            out=h3, in0=pt, scalar1=wb[:, 6:7], scalar2=0.0,
            op0=ALU.add, op1=ALU.max)
        pt = psum_mlp.tile([64, B_SH], f32, tag="mlp")
        nc.tensor.matmul(pt, w34[:, 256:320], h3, start=True, stop=True)
        nc.vector.tensor_scalar(
            out=hhg[0:64, :], in0=pt, scalar1=wb[0:64, 7:8], scalar2=0.0,
            op0=ALU.add, op1=ALU.max)

        # GRU hidden-path init matmuls (f32 operands, one-time).
        nc.tensor.matmul(prz, whh[:, 0:128], hhg, start=False, stop=False, **kw)
        nc.tensor.matmul(phn, whh[:, 128:192], hhg, start=True, stop=False, **kw)
        nc.tensor.matmul(pd1, ow1, hhg, start=True, stop=False, **kw)

        # --- GRU: persistent psum accumulators, 8 unrolled steps.
        # DVE runs the elementwise chain (PSUM-capable); Pool takes the
        # off-chain hh update; ACT does sigmoid/tanh.  The sigmoid output
        # lands in PSUM (cheaper ACT access), reusing the idle MLP banks.
        x_prev = None
        for t in range(T):
            last = t == T - 1
            rz = work.tile([128, B_SH], f32, tag="rz")
            nc.scalar.activation(rz, prz, AF.Sigmoid)
            tmp = work.tile([64, B_SH], f32, tag="tmp")
            nc.vector.tensor_mul(tmp, rz[0:64, :], phn)     # r * h_n
            ptm = psum_gru.tile([64, B_SH], f32, tag="ptm")
            nc.vector.tensor_add(ptm, tmp, pin)             # + i_n
            zc = work.tile([64, B_SH], f32, tag="zc")
            nc.gpsimd.tensor_scalar(
                out=zc, in0=rz[64:128, :], scalar1=-1.0, scalar2=1.0,
                op0=ALU.mult, op1=ALU.add)                  # 1 - z
            n_t = work.tile([64, B_SH], f32, tag="n_t")
            nc.scalar.activation(n_t, ptm, AF.Tanh)
            t1 = work.tile([64, B_SH], f32, tag="t1")
            nc.vector.tensor_sub(t1, hhg[0:64, :], n_t)     # hh - n
            dlt = work.tile([64, B_SH], f16, tag="dlt")
            nc.vector.tensor_mul(dlt, zc, t1)               # d = (1-z)(hh-n)

            # hh' = hh - d; pd1 first (it gates the output path); the
            # whhn updates are ready before the relu, so PE runs them
            # during the relu's sem latency.
            nc.tensor.matmul(pd1, ow1n, dlt,
                             start=False, stop=last, **kw)
            if not last:
                nc.tensor.matmul(prz, whhn[:, 0:128], dlt,
                                 start=False, stop=False, **kw)
                nc.tensor.matmul(phn, whhn[:, 128:192], dlt,
                                 start=False, stop=(t == T - 2), **kw)
                dlt32 = work.tile([64, B_SH], f32, tag="dlt32")
                nc.gpsimd.tensor_mul(dlt32, zc, t1)
                nc.gpsimd.tensor_sub(hhg[0:64, :], hhg[0:64, :], dlt32)
            nc.vector.tensor_scalar_max(d1g[0:4, :], pd1, 0.0)  # d1(hh')
            if not last:
                # x-recurrence folded through d1g
                nc.tensor.matmul(prz, wixo[:, 0:128], d1g,
                                 start=False, stop=(t == T - 2), **kw)
                nc.tensor.matmul(pin, wixo[:, 128:192], d1g,
                                 start=False, stop=(t == T - 2), **kw)

            # x output (off the critical chain)
            pd3 = psum_gru.tile([3, B_SH], f32, tag="pd3")
            nc.tensor.matmul(pd3, ow23, d1g, start=True, stop=True)
            x_new = xpool.tile([3, B_SH], f32, tag="x")
            if x_prev is None:
                nc.vector.tensor_copy(x_new, pd3)
            else:
                nc.vector.tensor_add(x_new, x_prev, pd3)
            nc.sync.dma_start(out=out_d[3 * t:3 * t + 3, :], in_=x_new)
            x_prev = x_new

    nc.compile()
    return nc


def _get_program():
    if "nc" not in _CACHE:
        _CACHE["nc"] = _build_program()
    return _CACHE["nc"]


def make_in_maps(**inputs) -> list[dict]:
    """Host-side packing + data-parallel sharding -> one in_map per core."""
    f = lambda a: np.ascontiguousarray(np.asarray(a, dtype=np.float32))
    z = f(inputs["z"]).reshape(B, C, S)
    gp = f(inputs["goal_point"])
    gps = f(inputs["goal_point_speed"])
    W_ih, W_hh = f(inputs["W_ih"]), f(inputs["W_hh"])
    b_ih, b_hh = f(inputs["b_ih"]), f(inputs["b_hh"])
    oW1, ob1 = f(inputs["oW1"]), f(inputs["ob1"])
    oW2, ob2 = f(inputs["oW2"]), f(inputs["ob2"])
    oW3, ob3 = f(inputs["oW3"]), f(inputs["ob3"])

    # layer-1 weight: fold the 1/S mean scale and the z-layout channel
    # permutation (chunk j, partition p <-> channel 4p+j).
    jw1t = f(inputs["jW1"]).T * np.float32(1.0 / S)
    perm = (4 * np.arange(128)[None, :] + np.arange(4)[:, None]).reshape(-1)
    w1 = np.ascontiguousarray(jw1t[perm]).astype(np.float16)
    w2 = np.ascontiguousarray(f(inputs["jW2"]).T).astype(np.float16)
    jw3t = f(inputs["jW3"]).T.astype(np.float16)                 # [256, 128]
    jw4t = f(inputs["jW4"]).T.astype(np.float16)                 # [128, 64]
    w34 = np.zeros((128, 320), np.float16)
    w34[:, 0:128] = jw3t[0:128]
    w34[:, 128:256] = jw3t[128:256]
    w34[:, 256:320] = jw4t

    # bias pack [128, 8]: jb1 (4 cols), jb2 (2), jb3 (1), jb4 (1, rows 0:64)
    wbias = np.zeros((128, 8), np.float32)
    wbias[:, 0:4] = f(inputs["jb1"]).reshape(4, 128).T
    wbias[:, 4:6] = f(inputs["jb2"]).reshape(2, 128).T
    wbias[:, 6] = f(inputs["jb3"])
    wbias[0:64, 7] = f(inputs["jb4"])

    brow = np.concatenate([b_ih[0:128] + b_hh[0:128], b_ih[128:192]])
    wgobt = np.concatenate([W_ih[:, 3:6].T, brow[None, :]])  # [4, 192]
    brow2 = np.concatenate([np.zeros(128, np.float32), b_hh[128:192]])
    whhbt = np.concatenate([W_hh.T, brow2[None, :]])         # [65, 192]
    ow1bt = np.concatenate([oW1.T, ob1[None, :]])            # [65, 4]

    w23 = oW2.T @ oW3.T                                      # [4, 3]
    b23 = ob2 @ oW3.T + ob3                                  # [3]
    wp16 = np.zeros((64, P16_COLS), np.float16)
    wp16[0:64, 0:192] = -W_hh.T
    wp16[0:4, 192:384] = w23 @ W_ih[:, 0:3].T
    wp16[32, 192:384] = W_ih[:, 0:3] @ b23
    wp16[0:64, 384:388] = -oW1.T
    wp16[0:4, 388:391] = w23
    wp16[32, 388:391] = b23

    goalT = np.stack([gp[:, 0, 3], gp[:, 1, 3], gps])        # [3, 256]

    z16 = z.astype(np.float16)

    in_maps = []
    for i in range(N_CORES):
        sl = slice(i * B_SH, (i + 1) * B_SH)
        wp32 = np.zeros((65, P32_COLS), np.float32)
        wp32[0:65, 0:192] = whhbt
        wp32[0:4, 192:384] = wgobt
        wp32[0:3, 384:384 + B_SH] = goalT[:, sl]
        wp32[3, 384:384 + B_SH] = 1.0
        wp32[0:65, 416:420] = ow1bt
        in_maps.append(dict(
            z=np.ascontiguousarray(z16[sl].reshape(ROWS, S)),
            w1=w1, w2=w2, w34=w34, wbias=wbias,
            wp32=wp32, wp16=wp16,
        ))
    return in_maps


def unshard_out(results: list[dict]) -> np.ndarray:
    # per-core out [24, 32]: row 3t+c, col b  ->  [32, 8, 3]
    parts = [r["out"].reshape(T, 3, B_SH).transpose(2, 0, 1) for r in results]
    return np.ascontiguousarray(np.concatenate(parts, axis=0), dtype=np.float32)


def kernel(**inputs) -> np.ndarray:
    from concourse.bass_utils import run_bass_kernel_spmd

    nc = _get_program()
    in_maps = make_in_maps(**inputs)
    res = run_bass_kernel_spmd(nc, in_maps, core_ids=list(range(N_CORES)))
    return unshard_out(res.results)


# revision 11
# speedup vs baseline: 1.3178x; 1.0633x over previous
"""Trainium2 Bass kernel for nn_ImitationHead (dense_mlp).

Computation (per batch row b of 256):
  h  = mean(z[b], spatial)                # [512] <- z [512,16,16]
  h  = relu-MLP chain 512->512->256->128->64
  goal = [goal_point[b,0,3], goal_point[b,1,3], goal_point_speed[b]]
  GRU (hidden 64, input [x(3); goal(3)]) unrolled 8 steps, each step
  followed by an output MLP 64->4(relu)->4->3 producing dx; x += dx.
  Output: the 8 x values -> [256, 8, 3].

Sharding: pure data parallel, batch 256 -> 8 cores x 32.

Key layout/perf choices (v2):
  - z and all join-MLP weights travel as float16: halves the HBM
    traffic that dominates the kernel (8 MiB z + 0.9 MiB weights per
    core).  fp16 keeps 10 mantissa bits so the 2e-2 tolerance is safe.
  - on-chip layout fully "transposed" (features on partitions, batch on
    the free axis); z shard viewed as [16384, 256] f16, 8 DMAs of
    [128p, 4, 4, 256] (1 MiB) with 2 KiB contiguous DRAM runs.  The
    channel permutation (chunk j, partition p <-> channel 4p+j) is
    undone by permuting the rows of the layer-1 weight on the host.
  - spatial-sum reduces are split DVE / Pool (j-outer so the first L1
    k-chunk can start right after the last DMA's first reduce group).
  - join MLP matmuls in fp16 (1 PE cycle/row instead of 4); bias+ReLU
    fused on the Pool engine via tensor_scalar with a per-partition
    bias AP (no ACT round-trip).
  - GRU: persistent PSUM accumulators; hh' = hh - d with
    d = (1-z)*(hh-n); the x-recurrence folds through the output MLP.
    All elementwise ops on Pool, sigmoid/tanh on ACT writing PSUM,
    per-step incremental matmuls in fp16.  Biases fold in as an extra
    all-ones input row at init; output 4->4 and 4->3 layers fold into
    one 4->3 matrix on the host; mean's 1/256 folds into W1.
"""

import numpy as np
from contextlib import ExitStack

N_CORES = 8
B = 256
B_SH = B // N_CORES       # 32 batch rows per core
C = 512                   # channels
S = 256                   # spatial 16*16
HID = 64
T = 8                     # pred_len
ROWS = B_SH * C           # 16384 z rows per core
N_DMA = 8                 # z DMAs per core (1 MiB f16 each)
H_PER = 4                 # batch blocks per z DMA
J = 4                     # 256-chunks per partition per batch block

# f32 constant pack [65, 420]
_OFF32 = dict(whhbt=0, wgobt=192, goalones=384, ow1bt=416)
P32_COLS = 420
# f16 constant pack [64, 391]
_OFF16 = dict(whhnbt=0, wixobt=192, ow1nbt=384, ow23bt=388)
P16_COLS = 391

_CACHE: dict = {}


def _build_program():
    import concourse.bacc as bacc
    import concourse.tile as tile
    from concourse import mybir

    f32 = mybir.dt.float32
    f16 = mybir.dt.float16
    AF = mybir.ActivationFunctionType
    AX = mybir.AxisListType
    ALU = mybir.AluOpType

    nc = bacc.Bacc("TRN2", target_bir_lowering=False, debug=False)

    z = nc.dram_tensor("z", [ROWS, S], f16, kind="ExternalInput")
    w1d = nc.dram_tensor("w1", [512, 512], f16, kind="ExternalInput")
    w2d = nc.dram_tensor("w2", [512, 256], f16, kind="ExternalInput")
    w34d = nc.dram_tensor("w34", [128, 320], f16, kind="ExternalInput")
    wbias_d = nc.dram_tensor("wbias", [128, 8], f32, kind="ExternalInput")
    wp32_d = nc.dram_tensor("wp32", [65, P32_COLS], f32, kind="ExternalInput")
    wp16_d = nc.dram_tensor("wp16", [64, P16_COLS], f16, kind="ExternalInput")
    out_d = nc.dram_tensor("out", [3 * T, B_SH], f32, kind="ExternalOutput")

    with tile.TileContext(nc) as tc, ExitStack() as ctx, \
            nc.allow_low_precision(reason="fp16 pipeline; output tol 2e-2"):
        consts = ctx.enter_context(tc.tile_pool(name="consts", bufs=1))
        zpool = ctx.enter_context(tc.tile_pool(name="zpool", bufs=3))
        hpool = ctx.enter_context(tc.tile_pool(name="hpool", bufs=1))
        work = ctx.enter_context(tc.tile_pool(name="work", bufs=2))
        xpool = ctx.enter_context(tc.tile_pool(name="xpool", bufs=2))
        psum_mlp = ctx.enter_context(
            tc.tile_pool(name="psum_mlp", bufs=2, space="PSUM"))
        psum_gru = ctx.enter_context(
            tc.tile_pool(name="psum_gru", bufs=1, space="PSUM"))

        # --- small constant loads first (init matmuls depend on them) ---
        wb = consts.tile([128, 8], f32)
        nc.sync.dma_start(out=wb, in_=wbias_d[:])
        wp32 = consts.tile([65, P32_COLS], f32)
        nc.sync.dma_start(out=wp32, in_=wp32_d[:])
        wp16 = consts.tile([64, P16_COLS], f16)
        nc.sync.dma_start(out=wp16, in_=wp16_d[:])

        whh = wp32[0:65, 0:192]
        wgo = wp32[0:4, 192:384]
        gl = wp32[0:4, 384:384 + B_SH]
        ow1 = wp32[0:65, 416:420]
        whhn = wp16[0:64, 0:192]
        wixo = wp16[0:33, 192:384]
        ow1n = wp16[0:64, 384:388]
        ow23 = wp16[0:33, 388:391]

        # ACT table warmup: sigmoid/tanh tables resident before the tail.
        warm = consts.tile([1, 1], f32)
        nc.vector.memset(warm, 0.0)
        nc.scalar.activation(warm, warm, AF.Sigmoid)
        nc.scalar.activation(warm, warm, AF.Tanh)

        # hhg rows 0:64 = GRU hidden state (in-place across steps), row 64 = 1.
        hhg = hpool.tile([65, B_SH], f32)
        nc.vector.memset(hhg[64:65, :], 1.0)
        # d1g: relu(pd1) with ones row at partition 32; rows 4:32 stay zero
        # so the K=33 matmuls see only d1 + bias.
        d1g = hpool.tile([33, B_SH], f16)
        nc.vector.memset(d1g[0:33, :], 0.0)
        nc.vector.memset(d1g[32:33, :], 1.0)

        # GRU goal-path init matmuls: depend only on wp32, run during z.
        kw = dict(skip_group_check=True)
        prz = psum_gru.tile([128, B_SH], f32, tag="prz")   # r/z pre-act
        pin = psum_gru.tile([64, B_SH], f32, tag="pin")    # i_n pre-act
        phn = psum_gru.tile([64, B_SH], f32, tag="phn")    # h_n pre-act
        pd1 = psum_gru.tile([4, B_SH], f32, tag="pd1")     # oW1@hh+ob1
        nc.tensor.matmul(prz, wgo[:, 0:128], gl, start=True, stop=False, **kw)
        nc.tensor.matmul(pin, wgo[:, 128:192], gl, start=True, stop=False, **kw)

        # --- z stream: 8 x 1MiB f16 DMAs.  TensorReduce runs at 1x on
        # DVE but TensorTensor fp16 gets the 2x perf mode, so the spatial
        # sum is mostly a binary tree of fp16 adds on DVE (rows h=0..2),
        # with the h=3 row's four [128,256] chunks split between ACT
        # (Copy+accum_out) and Pool (STT+accum_out) so each engine stays
        # under the 2.9us DMA time.  The last DMA is split in half so the
        # final tree only trails the stream by ~1.4us.
        # Row d*2048 + h*512 + 4p + j -> batch b = 4d+h, channel 4p+j.
        hTc = hpool.tile([128, B_SH, J], f16)
        junk_a = hpool.tile([128, S], f16)
        z_r = z[:].rearrange("(d h p j) s -> d p h j s", h=H_PER, p=128, j=J)

        def tree_reduce(src_ap, n_rows, out_ap):
            # src_ap [128, n_rows, S] f16 -> out_ap [128, n_rows, 1]:
            # 4 halving TT stages then one 1x multi-axis reduce of 16.
            scA = work.tile([128, n_rows, S // 2], f16, tag=f"trA{n_rows}")
            scB = work.tile([128, n_rows, S // 4], f16, tag=f"trB{n_rows}")
            n = S // 2
            nc.vector.tensor_add(scA[:, :, 0:n], src_ap[:, :, 0:n],
                                 src_ap[:, :, n:2 * n])
            cur, oth = scA, scB
            while n > 16:
                h_n = n // 2
                nc.vector.tensor_add(oth[:, :, 0:h_n], cur[:, :, 0:h_n],
                                     cur[:, :, h_n:n])
                cur, oth = oth, cur
                n = h_n
            nc.vector.tensor_reduce(out=out_ap, in_=cur[:, :, 0:16],
                                    axis=AX.X, op=ALU.add)

        def act_chunks(zt, h, b, js):
            for j in js:
                nc.scalar.activation(
                    out=junk_a, in_=zt[:, h, j, :], func=AF.Copy,
                    accum_out=hTc[:, b, j:j + 1])

        for d in range(N_DMA - 1):
            zt = zpool.tile([128, H_PER, J, S], f16, tag="zt")
            nc.sync.dma_start(out=zt, in_=z_r[d])
            b = H_PER * d
            tree_reduce(zt[:, 0:3].rearrange("p h j s -> p (h j) s"), 3 * J,
                        hTc[:, b:b + 3, :].rearrange("p b j -> p (b j) ()"))
            act_chunks(zt, 3, b + 3, range(4))
        d = N_DMA - 1
        for half in range(2):
            zh = zpool.tile([128, 2, J, S], f16, tag="zh")
            nc.sync.dma_start(out=zh, in_=z_r[d][:, 2 * half:2 * half + 2])
            b = H_PER * d + 2 * half
            # DVE: row 0 fully + half of row 1; ACT: the other half
            tree_reduce(
                zh[:].rearrange("p h j s -> p (h j) s")[:, 0:6, :], 6,
                hTc[:, b:b + 2, :].rearrange("p b j -> p (b j) ()")[:, 0:6, :])
            act_chunks(zh, 1, b + 1, range(2, 4))
        # --- MLP weights queued after the z stream (needed later) ---
        w1 = consts.tile([128, 4, 512], f16)
        jw1_r = w1d[:].rearrange("(k p) m -> k p m", p=128)
        for k in range(0, 4, 2):
            nc.sync.dma_start(out=w1[:, k:k + 2, :], in_=jw1_r[k:k + 2])
        w2 = consts.tile([128, 4, 256], f16)
        nc.sync.dma_start(out=w2, in_=w2d[:].rearrange("(k p) m -> p k m", p=128))
        w34 = consts.tile([128, 320], f16)
        nc.sync.dma_start(out=w34, in_=w34d[:])

        # --- join MLP (transposed): hN_T = relu(W @ h_T + b) ---
        # bias+relu fused on Pool: (psum + bias) max 0 -> f16
        h1 = hpool.tile([128, 4, B_SH], f16)
        for m in range(4):
            pt = psum_mlp.tile([128, B_SH], f32, tag="mlp")
            for k in range(4):
                nc.tensor.matmul(pt, w1[:, k, m * 128:(m + 1) * 128],
                                 hTc[:, :, k],
                                 start=(k == 0), stop=(k == 3))
            nc.vector.tensor_scalar(
                out=h1[:, m, :], in0=pt, scalar1=wb[:, m:m + 1], scalar2=0.0,
                op0=ALU.add, op1=ALU.max)
        h2 = hpool.tile([128, 2, B_SH], f16)
        for m in range(2):
            pt = psum_mlp.tile([128, B_SH], f32, tag="mlp")
            for k in range(4):
                nc.tensor.matmul(pt, w2[:, k, m * 128:(m + 1) * 128], h1[:, k, :],
                                 start=(k == 0), stop=(k == 3))
            nc.vector.tensor_scalar(
                out=h2[:, m, :], in0=pt, scalar1=wb[:, 4 + m:5 + m], scalar2=0.0,
                op0=ALU.add, op1=ALU.max)
        h3 = hpool.tile([128, B_SH], f16)
        pt = psum_mlp.tile([128, B_SH], f32, tag="mlp")
        for k in range(2):
            nc.tensor.matmul(pt, w34[:, k * 128:(k + 1) * 128], h2[:, k, :],
                             start=(k == 0), stop=(k == 1))
        nc.vector.tensor_scalar(
            out=h3, in0=pt, scalar1=wb[:, 6:7], scalar2=0.0,
            op0=ALU.add, op1=ALU.max)
        pt = psum_mlp.tile([64, B_SH], f32, tag="mlp")
        nc.tensor.matmul(pt, w34[:, 256:320], h3, start=True, stop=True)
        nc.vector.tensor_scalar(
            out=hhg[0:64, :], in0=pt, scalar1=wb[0:64, 7:8], scalar2=0.0,
            op0=ALU.add, op1=ALU.max)

        # GRU hidden-path init matmuls (f32 operands, one-time).
        nc.tensor.matmul(prz, whh[:, 0:128], hhg, start=False, stop=False, **kw)
        nc.tensor.matmul(phn, whh[:, 128:192], hhg, start=True, stop=False, **kw)
        nc.tensor.matmul(pd1, ow1, hhg, start=True, stop=False, **kw)

        # --- GRU: persistent psum accumulators, 8 unrolled steps.
        # DVE runs the elementwise chain (PSUM-capable); Pool takes the
        # off-chain hh update; ACT does sigmoid/tanh.  The sigmoid output
        # lands in PSUM (cheaper ACT access), reusing the idle MLP banks.
        x_prev = None
        for t in range(T):
            last = t == T - 1
            rz = work.tile([128, B_SH], f32, tag="rz")
            nc.scalar.activation(rz, prz, AF.Sigmoid)
            tmp = work.tile([64, B_SH], f32, tag="tmp")
            nc.vector.tensor_mul(tmp, rz[0:64, :], phn)     # r * h_n
            ptm = psum_gru.tile([64, B_SH], f32, tag="ptm")
            nc.vector.tensor_add(ptm, tmp, pin)             # + i_n
            zc = work.tile([64, B_SH], f32, tag="zc")
            nc.gpsimd.tensor_scalar(
                out=zc, in0=rz[64:128, :], scalar1=-1.0, scalar2=1.0,
                op0=ALU.mult, op1=ALU.add)                  # 1 - z
            n_t = work.tile([64, B_SH], f32, tag="n_t")
            nc.scalar.activation(n_t, ptm, AF.Tanh)
            t1 = work.tile([64, B_SH], f32, tag="t1")
            nc.vector.tensor_sub(t1, hhg[0:64, :], n_t)     # hh - n
            dlt = work.tile([64, B_SH], f16, tag="dlt")
            nc.vector.tensor_mul(dlt, zc, t1)               # d = (1-z)(hh-n)

            # hh' = hh - d; pd1 first (it gates the output path); the
            # whhn updates are ready before the relu, so PE runs them
            # during the relu's sem latency.
            nc.tensor.matmul(pd1, ow1n, dlt,
                             start=False, stop=last, **kw)
            if not last:
                nc.tensor.matmul(prz, whhn[:, 0:128], dlt,
                                 start=False, stop=False, **kw)
                nc.tensor.matmul(phn, whhn[:, 128:192], dlt,
                                 start=False, stop=(t == T - 2), **kw)
                dlt32 = work.tile([64, B_SH], f32, tag="dlt32")
                nc.gpsimd.tensor_mul(dlt32, zc, t1)
                nc.gpsimd.tensor_sub(hhg[0:64, :], hhg[0:64, :], dlt32)
            nc.vector.tensor_scalar_max(d1g[0:4, :], pd1, 0.0)  # d1(hh')
            if not last:
                # x-recurrence folded through d1g
                nc.tensor.matmul(prz, wixo[:, 0:128], d1g,
                                 start=False, stop=(t == T - 2), **kw)
                nc.tensor.matmul(pin, wixo[:, 128:192], d1g,
                                 start=False, stop=(t == T - 2), **kw)

            # x output (off the critical chain)
            pd3 = psum_gru.tile([3, B_SH], f32, tag="pd3")
            nc.tensor.matmul(pd3, ow23, d1g, start=True, stop=True)
            x_new = xpool.tile([3, B_SH], f32, tag="x")
            if x_prev is None:
                nc.vector.tensor_copy(x_new, pd3)
            else:
                nc.vector.tensor_add(x_new, x_prev, pd3)
            nc.sync.dma_start(out=out_d[3 * t:3 * t + 3, :], in_=x_new)
            x_prev = x_new

    nc.compile()
    return nc


def _get_program():
    if "nc" not in _CACHE:
        _CACHE["nc"] = _build_program()
    return _CACHE["nc"]


def make_in_maps(**inputs) -> list[dict]:
    """Host-side packing + data-parallel sharding -> one in_map per core."""
    f = lambda a: np.ascontiguousarray(np.asarray(a, dtype=np.float32))
    z = f(inputs["z"]).reshape(B, C, S)
    gp = f(inputs["goal_point"])
    gps = f(inputs["goal_point_speed"])
    W_ih, W_hh = f(inputs["W_ih"]), f(inputs["W_hh"])
    b_ih, b_hh = f(inputs["b_ih"]), f(inputs["b_hh"])
    oW1, ob1 = f(inputs["oW1"]), f(inputs["ob1"])
    oW2, ob2 = f(inputs["oW2"]), f(inputs["ob2"])
    oW3, ob3 = f(inputs["oW3"]), f(inputs["ob3"])

    # layer-1 weight: fold the 1/S mean scale and the z-layout channel
    # permutation (chunk j, partition p <-> channel 4p+j).
    jw1t = f(inputs["jW1"]).T * np.float32(1.0 / S)
    perm = (4 * np.arange(128)[None, :] + np.arange(4)[:, None]).reshape(-1)
    w1 = np.ascontiguousarray(jw1t[perm]).astype(np.float16)
    w2 = np.ascontiguousarray(f(inputs["jW2"]).T).astype(np.float16)
    jw3t = f(inputs["jW3"]).T.astype(np.float16)                 # [256, 128]
    jw4t = f(inputs["jW4"]).T.astype(np.float16)                 # [128, 64]
    w34 = np.zeros((128, 320), np.float16)
    w34[:, 0:128] = jw3t[0:128]
    w34[:, 128:256] = jw3t[128:256]
    w34[:, 256:320] = jw4t

    # bias pack [128, 8]: jb1 (4 cols), jb2 (2), jb3 (1), jb4 (1, rows 0:64)
    wbias = np.zeros((128, 8), np.float32)
    wbias[:, 0:4] = f(inputs["jb1"]).reshape(4, 128).T
    wbias[:, 4:6] = f(inputs["jb2"]).reshape(2, 128).T
    wbias[:, 6] = f(inputs["jb3"])
    wbias[0:64, 7] = f(inputs["jb4"])

    brow = np.concatenate([b_ih[0:128] + b_hh[0:128], b_ih[128:192]])
    wgobt = np.concatenate([W_ih[:, 3:6].T, brow[None, :]])  # [4, 192]
    brow2 = np.concatenate([np.zeros(128, np.float32), b_hh[128:192]])
    whhbt = np.concatenate([W_hh.T, brow2[None, :]])         # [65, 192]
    ow1bt = np.concatenate([oW1.T, ob1[None, :]])            # [65, 4]

    w23 = oW2.T @ oW3.T                                      # [4, 3]
    b23 = ob2 @ oW3.T + ob3                                  # [3]
    wp16 = np.zeros((64, P16_COLS), np.float16)
    wp16[0:64, 0:192] = -W_hh.T
    wp16[0:4, 192:384] = w23 @ W_ih[:, 0:3].T
    wp16[32, 192:384] = W_ih[:, 0:3] @ b23
    wp16[0:64, 384:388] = -oW1.T
    wp16[0:4, 388:391] = w23
    wp16[32, 388:391] = b23

    goalT = np.stack([gp[:, 0, 3], gp[:, 1, 3], gps])        # [3, 256]

    z16 = z.astype(np.float16)

    in_maps = []
    for i in range(N_CORES):
        sl = slice(i * B_SH, (i + 1) * B_SH)
        wp32 = np.zeros((65, P32_COLS), np.float32)
        wp32[0:65, 0:192] = whhbt
        wp32[0:4, 192:384] = wgobt
        wp32[0:3, 384:384 + B_SH] = goalT[:, sl]
        wp32[3, 384:384 + B_SH] = 1.0
        wp32[0:65, 416:420] = ow1bt
        in_maps.append(dict(
            z=np.ascontiguousarray(z16[sl].reshape(ROWS, S)),
            w1=w1, w2=w2, w34=w34, wbias=wbias,
            wp32=wp32, wp16=wp16,
        ))
    return in_maps


def unshard_out(results: list[dict]) -> np.ndarray:
    # per-core out [24, 32]: row 3t+c, col b  ->  [32, 8, 3]
    parts = [r["out"].reshape(T, 3, B_SH).transpose(2, 0, 1) for r in results]
    return np.ascontiguousarray(np.concatenate(parts, axis=0), dtype=np.float32)


def kernel(**inputs) -> np.ndarray:
    from concourse.bass_utils import run_bass_kernel_spmd

    nc = _get_program()
    in_maps = make_in_maps(**inputs)
    res = run_bass_kernel_spmd(nc, in_maps, core_ids=list(range(N_CORES)))
    return unshard_out(res.results)


# revision 14
# speedup vs baseline: 1.3551x; 1.0283x over previous
"""Trainium2 Bass kernel for nn_ImitationHead (dense_mlp).

Computation (per batch row b of 256):
  h  = mean(z[b], spatial)                # [512] <- z [512,16,16]
  h  = relu-MLP chain 512->512->256->128->64
  goal = [goal_point[b,0,3], goal_point[b,1,3], goal_point_speed[b]]
  GRU (hidden 64, input [x(3); goal(3)]) unrolled 8 steps, each step
  followed by an output MLP 64->4(relu)->4->3 producing dx; x += dx.
  Output: the 8 x values -> [256, 8, 3].

Sharding: pure data parallel, batch 256 -> 8 cores x 32.

Key layout/perf choices (v2):
  - z and all join-MLP weights travel as float16: halves the HBM
    traffic that dominates the kernel (8 MiB z + 0.9 MiB weights per
    core).  fp16 keeps 10 mantissa bits so the 2e-2 tolerance is safe.
  - on-chip layout fully "transposed" (features on partitions, batch on
    the free axis); z shard viewed as [16384, 256] f16, 8 DMAs of
    [128p, 4, 4, 256] (1 MiB) with 2 KiB contiguous DRAM runs.  The
    channel permutation (chunk j, partition p <-> channel 4p+j) is
    undone by permuting the rows of the layer-1 weight on the host.
  - spatial-sum reduces are split DVE / Pool (j-outer so the first L1
    k-chunk can start right after the last DMA's first reduce group).
  - join MLP matmuls in fp16 (1 PE cycle/row instead of 4); bias+ReLU
    fused on the Pool engine via tensor_scalar with a per-partition
    bias AP (no ACT round-trip).
  - GRU: persistent PSUM accumulators; hh' = hh - d with
    d = (1-z)*(hh-n); the x-recurrence folds through the output MLP.
    All elementwise ops on Pool, sigmoid/tanh on ACT writing PSUM,
    per-step incremental matmuls in fp16.  Biases fold in as an extra
    all-ones input row at init; output 4->4 and 4->3 layers fold into
    one 4->3 matrix on the host; mean's 1/256 folds into W1.
"""

import numpy as np
from contextlib import ExitStack

N_CORES = 8
B = 256
B_SH = B // N_CORES       # 32 batch rows per core
C = 512                   # channels
S = 256                   # spatial 16*16
HID = 64
T = 8                     # pred_len
ROWS = B_SH * C           # 16384 z rows per core
N_DMA = 8                 # z DMAs per core (1 MiB f16 each)
H_PER = 4                 # batch blocks per z DMA
J = 4                     # 256-chunks per partition per batch block

# f32 constant pack [65, 420]
_OFF32 = dict(whhbt=0, wgobt=192, goalones=384, ow1bt=416)
P32_COLS = 420
# f16 constant pack [64, 391]
_OFF16 = dict(whhnbt=0, wixobt=192, ow1nbt=384, ow23bt=388)
P16_COLS = 391

_CACHE: dict = {}


def _build_program():
    import concourse.bacc as bacc
    import concourse.tile as tile
    from concourse import mybir

    f32 = mybir.dt.float32
    f16 = mybir.dt.float16
    AF = mybir.ActivationFunctionType
    AX = mybir.AxisListType
    ALU = mybir.AluOpType

    nc = bacc.Bacc("TRN2", target_bir_lowering=False, debug=False)

    z = nc.dram_tensor("z", [ROWS, S], f16, kind="ExternalInput")
    w1d = nc.dram_tensor("w1", [512, 512], f16, kind="ExternalInput")
    w2d = nc.dram_tensor("w2", [512, 256], f16, kind="ExternalInput")
    w34d = nc.dram_tensor("w34", [128, 320], f16, kind="ExternalInput")
    wbias_d = nc.dram_tensor("wbias", [128, 8], f32, kind="ExternalInput")
    wp32_d = nc.dram_tensor("wp32", [65, P32_COLS], f32, kind="ExternalInput")
    wp16_d = nc.dram_tensor("wp16", [64, P16_COLS], f16, kind="ExternalInput")
    out_d = nc.dram_tensor("out", [3 * T, B_SH], f32, kind="ExternalOutput")

    with tile.TileContext(nc) as tc, ExitStack() as ctx, \
            nc.allow_low_precision(reason="fp16 pipeline; output tol 2e-2"):
        consts = ctx.enter_context(tc.tile_pool(name="consts", bufs=1))
        zpool = ctx.enter_context(tc.tile_pool(name="zpool", bufs=3))
        hpool = ctx.enter_context(tc.tile_pool(name="hpool", bufs=1))
        work = ctx.enter_context(tc.tile_pool(name="work", bufs=2))
        xpool = ctx.enter_context(tc.tile_pool(name="xpool", bufs=2))
        psum_mlp = ctx.enter_context(
            tc.tile_pool(name="psum_mlp", bufs=2, space="PSUM"))
        psum_gru = ctx.enter_context(
            tc.tile_pool(name="psum_gru", bufs=1, space="PSUM"))

        # --- small constant loads first (init matmuls depend on them) ---
        wb = consts.tile([128, 8], f32)
        nc.sync.dma_start(out=wb, in_=wbias_d[:])
        wp32 = consts.tile([65, P32_COLS], f32)
        nc.sync.dma_start(out=wp32, in_=wp32_d[:])
        wp16 = consts.tile([64, P16_COLS], f16)
        nc.sync.dma_start(out=wp16, in_=wp16_d[:])

        whh = wp32[0:65, 0:192]
        wgo = wp32[0:4, 192:384]
        gl = wp32[0:4, 384:384 + B_SH]
        ow1 = wp32[0:65, 416:420]
        whhn = wp16[0:64, 0:192]
        wixo = wp16[0:33, 192:384]
        ow1n = wp16[0:64, 384:388]
        ow23 = wp16[0:33, 388:391]

        # ACT table warmup: sigmoid/tanh tables resident before the tail.
        warm = consts.tile([1, 1], f32)
        nc.vector.memset(warm, 0.0)
        nc.scalar.activation(warm, warm, AF.Sigmoid)
        nc.scalar.activation(warm, warm, AF.Tanh)

        # hhg rows 0:64 = GRU hidden state (in-place across steps), row 64 = 1.
        hhg = hpool.tile([65, B_SH], f32)
        nc.vector.memset(hhg[64:65, :], 1.0)
        # d1g: relu(pd1) with ones row at partition 32; rows 4:32 stay zero
        # so the K=33 matmuls see only d1 + bias.
        d1g = hpool.tile([33, B_SH], f16)
        nc.vector.memset(d1g[0:33, :], 0.0)
        nc.vector.memset(d1g[32:33, :], 1.0)

        # GRU goal-path init matmuls: depend only on wp32, run during z.
        kw = dict(skip_group_check=True)
        prz = psum_gru.tile([128, B_SH], f32, tag="prz")   # r/z pre-act
        pin = psum_gru.tile([64, B_SH], f32, tag="pin")    # i_n pre-act
        phn = psum_gru.tile([64, B_SH], f32, tag="phn")    # h_n pre-act
        pd1 = psum_gru.tile([4, B_SH], f32, tag="pd1")     # oW1@hh+ob1
        nc.tensor.matmul(prz, wgo[:, 0:128], gl, start=True, stop=False, **kw)
        nc.tensor.matmul(pin, wgo[:, 128:192], gl, start=True, stop=False, **kw)

        # --- z stream: 8 x 1MiB f16 DMAs.  TensorReduce runs at 1x on
        # DVE but TensorTensor fp16 gets the 2x perf mode, so the spatial
        # sum is mostly a binary tree of fp16 adds on DVE (rows h=0..2),
        # with the h=3 row's four [128,256] chunks split between ACT
        # (Copy+accum_out) and Pool (STT+accum_out) so each engine stays
        # under the 2.9us DMA time.  The last DMA is split in half so the
        # final tree only trails the stream by ~1.4us.
        # Row d*2048 + h*512 + 4p + j -> batch b = 4d+h, channel 4p+j.
        hTc = hpool.tile([128, B_SH, J], f16)
        junk_a = hpool.tile([128, S], f16)
        z_r = z[:].rearrange("(d h p j) s -> d p h j s", h=H_PER, p=128, j=J)

        def tree_reduce(src_ap, n_rows, out_ap):
            # src_ap [128, n_rows, S] f16 -> out_ap [128, n_rows, 1]:
            # 4 halving TT stages then one 1x multi-axis reduce of 16.
            scA = work.tile([128, n_rows, S // 2], f16, tag=f"trA{n_rows}")
            scB = work.tile([128, n_rows, S // 4], f16, tag=f"trB{n_rows}")
            n = S // 2
            nc.vector.tensor_add(scA[:, :, 0:n], src_ap[:, :, 0:n],
                                 src_ap[:, :, n:2 * n])
            cur, oth = scA, scB
            while n > 16:
                h_n = n // 2
                nc.vector.tensor_add(oth[:, :, 0:h_n], cur[:, :, 0:h_n],
                                     cur[:, :, h_n:n])
                cur, oth = oth, cur
                n = h_n
            nc.vector.tensor_reduce(out=out_ap, in_=cur[:, :, 0:16],
                                    axis=AX.X, op=ALU.add)

        def act_chunks(zt, h, b, js):
            for j in js:
                nc.scalar.activation(
                    out=junk_a, in_=zt[:, h, j, :], func=AF.Copy,
                    accum_out=hTc[:, b, j:j + 1])

        for d in range(N_DMA - 1):
            zt = zpool.tile([128, H_PER, J, S], f16, tag="zt")
            nc.sync.dma_start(out=zt, in_=z_r[d])
            b = H_PER * d
            tree_reduce(zt[:, 0:3].rearrange("p h j s -> p (h j) s"), 3 * J,
                        hTc[:, b:b + 3, :].rearrange("p b j -> p (b j) ()"))
            act_chunks(zt, 3, b + 3, range(4))
        d = N_DMA - 1
        for half in range(2):
            zh = zpool.tile([128, 2, J, S], f16, tag="zh")
            nc.sync.dma_start(out=zh, in_=z_r[d][:, 2 * half:2 * half + 2])
            b = H_PER * d + 2 * half
            # DVE: row 0 fully + half of row 1; ACT: the other half
            tree_reduce(
                zh[:].rearrange("p h j s -> p (h j) s")[:, 0:6, :], 6,
                hTc[:, b:b + 2, :].rearrange("p b j -> p (b j) ()")[:, 0:6, :])
            act_chunks(zh, 1, b + 1, range(2, 4))
        # --- MLP weights queued after the z stream (needed later) ---
        w1 = consts.tile([128, 4, 512], f16)
        jw1_r = w1d[:].rearrange("(k p) m -> k p m", p=128)
        for k in range(0, 4, 2):
            nc.sync.dma_start(out=w1[:, k:k + 2, :], in_=jw1_r[k:k + 2])
        w2 = consts.tile([128, 4, 256], f16)
        nc.sync.dma_start(out=w2, in_=w2d[:].rearrange("(k p) m -> p k m", p=128))
        w34 = consts.tile([128, 320], f16)
        nc.sync.dma_start(out=w34, in_=w34d[:])

        # --- join MLP (transposed): hN_T = relu(W @ h_T + b) ---
        # bias+relu fused on Pool: (psum + bias) max 0 -> f16
        h1 = hpool.tile([128, 4, B_SH], f16)
        for m in range(4):
            pt = psum_mlp.tile([128, B_SH], f32, tag="mlp")
            for k in range(4):
                nc.tensor.matmul(pt, w1[:, k, m * 128:(m + 1) * 128],
                                 hTc[:, :, k],
                                 start=(k == 0), stop=(k == 3))
            nc.vector.tensor_scalar(
                out=h1[:, m, :], in0=pt, scalar1=wb[:, m:m + 1], scalar2=0.0,
                op0=ALU.add, op1=ALU.max)
        h2 = hpool.tile([128, 2, B_SH], f16)
        for m in range(2):
            pt = psum_mlp.tile([128, B_SH], f32, tag="mlp")
            for k in range(4):
                nc.tensor.matmul(pt, w2[:, k, m * 128:(m + 1) * 128], h1[:, k, :],
                                 start=(k == 0), stop=(k == 3))
            nc.vector.tensor_scalar(
                out=h2[:, m, :], in0=pt, scalar1=wb[:, 4 + m:5 + m], scalar2=0.0,
                op0=ALU.add, op1=ALU.max)
        h3 = hpool.tile([128, B_SH], f16)
        pt = psum_mlp.tile([128, B_SH], f32, tag="mlp")
        for k in range(2):
            nc.tensor.matmul(pt, w34[:, k * 128:(k + 1) * 128], h2[:, k, :],
                             start=(k == 0), stop=(k == 1))
        nc.vector.tensor_scalar(
            out=h3, in0=pt, scalar1=wb[:, 6:7], scalar2=0.0,
            op0=ALU.add, op1=ALU.max)
        pt = psum_mlp.tile([64, B_SH], f32, tag="mlp")
        nc.tensor.matmul(pt, w34[:, 256:320], h3, start=True, stop=True)
        nc.vector.tensor_scalar(
            out=hhg[0:64, :], in0=pt, scalar1=wb[0:64, 7:8], scalar2=0.0,
            op0=ALU.add, op1=ALU.max)

        # GRU hidden-path init matmuls (f32 operands, one-time).
        nc.tensor.matmul(prz, whh[:, 0:128], hhg, start=False, stop=False, **kw)
        nc.tensor.matmul(phn, whh[:, 128:192], hhg, start=True, stop=False, **kw)
        nc.tensor.matmul(pd1, ow1, hhg, start=True, stop=False, **kw)

        amr_junk = hpool.tile([64, 1], f32)
        # --- GRU: persistent psum accumulators, 8 unrolled steps.
        # DVE runs the elementwise chain (PSUM-capable); Pool takes the
        # off-chain hh update; ACT does sigmoid/tanh.  The sigmoid output
        # lands in PSUM (cheaper ACT access), reusing the idle MLP banks.
        x_prev = None
        for t in range(T):
            last = t == T - 1
            r_t = work.tile([64, B_SH], f32, tag="r_t")
            nc.scalar.activation(r_t, prz[0:64, :], AF.Sigmoid)
            z_t = work.tile([64, B_SH], f32, tag="z_t")
            nc.scalar.activation(z_t, prz[64:128, :], AF.Sigmoid)
            tmp = work.tile([64, B_SH], f32, tag="tmp")
            nc.vector.tensor_mul(tmp, r_t, phn)             # r * h_n
            ptm = psum_gru.tile([64, B_SH], f32, tag="ptm")
            nc.vector.tensor_add(ptm, tmp, pin)             # + i_n
            n_t = work.tile([64, B_SH], f32, tag="n_t")
            nc.scalar.activation(n_t, ptm, AF.Tanh)
            t1 = work.tile([64, B_SH], f32, tag="t1")
            nc.vector.tensor_sub(t1, hhg[0:64, :], n_t)     # hh - n
            # d = (1-z)*(hh-n) in one fused DVE op
            dlt = work.tile([64, B_SH], f16, tag="dlt")
            nc.vector.affine_mul_reduce(
                out=dlt, accum_out=amr_junk, in0=z_t, in1=t1,
                scale=-1.0, bias=1.0)

            # hh' = hh - d; pd1 first (it gates the output path); the
            # whhn updates are ready before the relu, so PE runs them
            # during the relu's sem latency.
            nc.tensor.matmul(pd1, ow1n, dlt,
                             start=False, stop=last, **kw)
            if not last:
                nc.tensor.matmul(prz, whhn[:, 0:128], dlt,
                                 start=False, stop=False, **kw)
                nc.tensor.matmul(phn, whhn[:, 128:192], dlt,
                                 start=False, stop=(t == T - 2), **kw)
                dlt32 = work.tile([64, B_SH], f32, tag="dlt32")
                nc.vector.affine_mul_reduce(
                    out=dlt32, accum_out=amr_junk, in0=z_t, in1=t1,
                    scale=-1.0, bias=1.0)
                nc.gpsimd.tensor_sub(hhg[0:64, :], hhg[0:64, :], dlt32)
            nc.vector.tensor_scalar_max(d1g[0:4, :], pd1, 0.0)  # d1(hh')
            if not last:
                # x-recurrence folded through d1g
                nc.tensor.matmul(prz, wixo[:, 0:128], d1g,
                                 start=False, stop=(t == T - 2), **kw)
                nc.tensor.matmul(pin, wixo[:, 128:192], d1g,
                                 start=False, stop=(t == T - 2), **kw)

            # x output (off the critical chain)
            pd3 = psum_gru.tile([3, B_SH], f32, tag="pd3")
            nc.tensor.matmul(pd3, ow23, d1g, start=True, stop=True)
            x_new = xpool.tile([3, B_SH], f32, tag="x")
            if x_prev is None:
                nc.vector.tensor_copy(x_new, pd3)
            else:
                nc.vector.tensor_add(x_new, x_prev, pd3)
            nc.sync.dma_start(out=out_d[3 * t:3 * t + 3, :], in_=x_new)
            x_prev = x_new

    nc.compile()
    return nc


def _get_program():
    if "nc" not in _CACHE:
        _CACHE["nc"] = _build_program()
    return _CACHE["nc"]


def make_in_maps(**inputs) -> list[dict]:
    """Host-side packing + data-parallel sharding -> one in_map per core."""
    f = lambda a: np.ascontiguousarray(np.asarray(a, dtype=np.float32))
    z = f(inputs["z"]).reshape(B, C, S)
    gp = f(inputs["goal_point"])
    gps = f(inputs["goal_point_speed"])
    W_ih, W_hh = f(inputs["W_ih"]), f(inputs["W_hh"])
    b_ih, b_hh = f(inputs["b_ih"]), f(inputs["b_hh"])
    oW1, ob1 = f(inputs["oW1"]), f(inputs["ob1"])
    oW2, ob2 = f(inputs["oW2"]), f(inputs["ob2"])
    oW3, ob3 = f(inputs["oW3"]), f(inputs["ob3"])

    # layer-1 weight: fold the 1/S mean scale and the z-layout channel
    # permutation (chunk j, partition p <-> channel 4p+j).
    jw1t = f(inputs["jW1"]).T * np.float32(1.0 / S)
    perm = (4 * np.arange(128)[None, :] + np.arange(4)[:, None]).reshape(-1)
    w1 = np.ascontiguousarray(jw1t[perm]).astype(np.float16)
    w2 = np.ascontiguousarray(f(inputs["jW2"]).T).astype(np.float16)
    jw3t = f(inputs["jW3"]).T.astype(np.float16)                 # [256, 128]
    jw4t = f(inputs["jW4"]).T.astype(np.float16)                 # [128, 64]
    w34 = np.zeros((128, 320), np.float16)
    w34[:, 0:128] = jw3t[0:128]
    w34[:, 128:256] = jw3t[128:256]
    w34[:, 256:320] = jw4t

    # bias pack [128, 8]: jb1 (4 cols), jb2 (2), jb3 (1), jb4 (1, rows 0:64)
    wbias = np.zeros((128, 8), np.float32)
    wbias[:, 0:4] = f(inputs["jb1"]).reshape(4, 128).T
    wbias[:, 4:6] = f(inputs["jb2"]).reshape(2, 128).T
    wbias[:, 6] = f(inputs["jb3"])
    wbias[0:64, 7] = f(inputs["jb4"])

    brow = np.concatenate([b_ih[0:128] + b_hh[0:128], b_ih[128:192]])
    wgobt = np.concatenate([W_ih[:, 3:6].T, brow[None, :]])  # [4, 192]
    brow2 = np.concatenate([np.zeros(128, np.float32), b_hh[128:192]])
    whhbt = np.concatenate([W_hh.T, brow2[None, :]])         # [65, 192]
    ow1bt = np.concatenate([oW1.T, ob1[None, :]])            # [65, 4]

    w23 = oW2.T @ oW3.T                                      # [4, 3]
    b23 = ob2 @ oW3.T + ob3                                  # [3]
    wp16 = np.zeros((64, P16_COLS), np.float16)
    wp16[0:64, 0:192] = -W_hh.T
    wp16[0:4, 192:384] = w23 @ W_ih[:, 0:3].T
    wp16[32, 192:384] = W_ih[:, 0:3] @ b23
    wp16[0:64, 384:388] = -oW1.T
    wp16[0:4, 388:391] = w23
    wp16[32, 388:391] = b23

    goalT = np.stack([gp[:, 0, 3], gp[:, 1, 3], gps])        # [3, 256]

    z16 = z.astype(np.float16)

    in_maps = []
    for i in range(N_CORES):
        sl = slice(i * B_SH, (i + 1) * B_SH)
        wp32 = np.zeros((65, P32_COLS), np.float32)
        wp32[0:65, 0:192] = whhbt
        wp32[0:4, 192:384] = wgobt
        wp32[0:3, 384:384 + B_SH] = goalT[:, sl]
        wp32[3, 384:384 + B_SH] = 1.0
        wp32[0:65, 416:420] = ow1bt
        in_maps.append(dict(
            z=np.ascontiguousarray(z16[sl].reshape(ROWS, S)),
            w1=w1, w2=w2, w34=w34, wbias=wbias,
            wp32=wp32, wp16=wp16,
        ))
    return in_maps


def unshard_out(results: list[dict]) -> np.ndarray:
    # per-core out [24, 32]: row 3t+c, col b  ->  [32, 8, 3]
    parts = [r["out"].reshape(T, 3, B_SH).transpose(2, 0, 1) for r in results]
    return np.ascontiguousarray(np.concatenate(parts, axis=0), dtype=np.float32)


def kernel(**inputs) -> np.ndarray:
    from concourse.bass_utils import run_bass_kernel_spmd

    nc = _get_program()
    in_maps = make_in_maps(**inputs)
    res = run_bass_kernel_spmd(nc, in_maps, core_ids=list(range(N_CORES)))
    return unshard_out(res.results)


# revision 15
# speedup vs baseline: 1.3564x; 1.0010x over previous
"""Trainium2 Bass kernel for nn_ImitationHead (dense_mlp).

Computation (per batch row b of 256):
  h  = mean(z[b], spatial)                # [512] <- z [512,16,16]
  h  = relu-MLP chain 512->512->256->128->64
  goal = [goal_point[b,0,3], goal_point[b,1,3], goal_point_speed[b]]
  GRU (hidden 64, input [x(3); goal(3)]) unrolled 8 steps, each step
  followed by an output MLP 64->4(relu)->4->3 producing dx; x += dx.
  Output: the 8 x values -> [256, 8, 3].

Sharding: pure data parallel, batch 256 -> 8 cores x 32.

Key layout/perf choices (v2):
  - z and all join-MLP weights travel as float16: halves the HBM
    traffic that dominates the kernel (8 MiB z + 0.9 MiB weights per
    core).  fp16 keeps 10 mantissa bits so the 2e-2 tolerance is safe.
  - on-chip layout fully "transposed" (features on partitions, batch on
    the free axis); z shard viewed as [16384, 256] f16, 8 DMAs of
    [128p, 4, 4, 256] (1 MiB) with 2 KiB contiguous DRAM runs.  The
    channel permutation (chunk j, partition p <-> channel 4p+j) is
    undone by permuting the rows of the layer-1 weight on the host.
  - spatial-sum reduces are split DVE / Pool (j-outer so the first L1
    k-chunk can start right after the last DMA's first reduce group).
  - join MLP matmuls in fp16 (1 PE cycle/row instead of 4); bias+ReLU
    fused on the Pool engine via tensor_scalar with a per-partition
    bias AP (no ACT round-trip).
  - GRU: persistent PSUM accumulators; hh' = hh - d with
    d = (1-z)*(hh-n); the x-recurrence folds through the output MLP.
    All elementwise ops on Pool, sigmoid/tanh on ACT writing PSUM,
    per-step incremental matmuls in fp16.  Biases fold in as an extra
    all-ones input row at init; output 4->4 and 4->3 layers fold into
    one 4->3 matrix on the host; mean's 1/256 folds into W1.
"""

import numpy as np
from contextlib import ExitStack

N_CORES = 8
B = 256
B_SH = B // N_CORES       # 32 batch rows per core
C = 512                   # channels
S = 256                   # spatial 16*16
HID = 64
T = 8                     # pred_len
ROWS = B_SH * C           # 16384 z rows per core
N_DMA = 8                 # z DMAs per core (1 MiB f16 each)
H_PER = 4                 # batch blocks per z DMA
J = 4                     # 256-chunks per partition per batch block

# f32 constant pack [65, 420]
_OFF32 = dict(whhbt=0, wgobt=192, goalones=384, ow1bt=416)
P32_COLS = 420
# f16 constant pack [64, 391]
_OFF16 = dict(whhnbt=0, wixobt=192, ow1nbt=384, ow23bt=388)
P16_COLS = 391

_CACHE: dict = {}


def _build_program():
    import concourse.bacc as bacc
    import concourse.tile as tile
    from concourse import mybir

    f32 = mybir.dt.float32
    f16 = mybir.dt.float16
    AF = mybir.ActivationFunctionType
    AX = mybir.AxisListType
    ALU = mybir.AluOpType

    nc = bacc.Bacc("TRN2", target_bir_lowering=False, debug=False)

    z = nc.dram_tensor("z", [ROWS, S], f16, kind="ExternalInput")
    w1d = nc.dram_tensor("w1", [512, 512], f16, kind="ExternalInput")
    w2d = nc.dram_tensor("w2", [512, 256], f16, kind="ExternalInput")
    w34d = nc.dram_tensor("w34", [128, 320], f16, kind="ExternalInput")
    wbias_d = nc.dram_tensor("wbias", [128, 8], f32, kind="ExternalInput")
    wp32_d = nc.dram_tensor("wp32", [65, P32_COLS], f32, kind="ExternalInput")
    wp16_d = nc.dram_tensor("wp16", [64, P16_COLS], f16, kind="ExternalInput")
    out_d = nc.dram_tensor("out", [3 * T, B_SH], f32, kind="ExternalOutput")

    with tile.TileContext(nc) as tc, ExitStack() as ctx, \
            nc.allow_low_precision(reason="fp16 pipeline; output tol 2e-2"):
        consts = ctx.enter_context(tc.tile_pool(name="consts", bufs=1))
        zpool = ctx.enter_context(tc.tile_pool(name="zpool", bufs=3))
        hpool = ctx.enter_context(tc.tile_pool(name="hpool", bufs=1))
        work = ctx.enter_context(tc.tile_pool(name="work", bufs=2))
        xpool = ctx.enter_context(tc.tile_pool(name="xpool", bufs=2))
        psum_mlp = ctx.enter_context(
            tc.tile_pool(name="psum_mlp", bufs=2, space="PSUM"))
        psum_gru = ctx.enter_context(
            tc.tile_pool(name="psum_gru", bufs=1, space="PSUM"))

        # --- small constant loads first (init matmuls depend on them) ---
        wb = consts.tile([128, 8], f32)
        nc.sync.dma_start(out=wb, in_=wbias_d[:])
        wp32 = consts.tile([65, P32_COLS], f32)
        nc.sync.dma_start(out=wp32, in_=wp32_d[:])
        wp16 = consts.tile([64, P16_COLS], f16)
        nc.sync.dma_start(out=wp16, in_=wp16_d[:])

        whh = wp32[0:65, 0:192]
        wgo = wp32[0:4, 192:384]
        gl = wp32[0:4, 384:384 + B_SH]
        ow1 = wp32[0:65, 416:420]
        whhn = wp16[0:64, 0:192]
        wixo = wp16[0:33, 192:384]
        ow1n = wp16[0:64, 384:388]
        ow23 = wp16[0:33, 388:391]

        # ACT table warmup: sigmoid/tanh tables resident before the tail.
        warm = consts.tile([1, 1], f32)
        nc.vector.memset(warm, 0.0)
        nc.scalar.activation(warm, warm, AF.Sigmoid)
        nc.scalar.activation(warm, warm, AF.Tanh)

        # hhg rows 0:64 = GRU hidden state (in-place across steps), row 64 = 1.
        hhg = hpool.tile([65, B_SH], f32)
        nc.vector.memset(hhg[64:65, :], 1.0)
        # d1g: relu(pd1) with ones row at partition 32; rows 4:32 stay zero
        # so the K=33 matmuls see only d1 + bias.
        d1g = hpool.tile([33, B_SH], f16)
        nc.vector.memset(d1g[0:33, :], 0.0)
        nc.vector.memset(d1g[32:33, :], 1.0)

        # GRU goal-path init matmuls: depend only on wp32, run during z.
        kw = dict(skip_group_check=True)
        prz = psum_gru.tile([128, B_SH], f32, tag="prz")   # r/z pre-act
        pin = psum_gru.tile([64, B_SH], f32, tag="pin")    # i_n pre-act
        phn = psum_gru.tile([64, B_SH], f32, tag="phn")    # h_n pre-act
        pd1 = psum_gru.tile([4, B_SH], f32, tag="pd1")     # oW1@hh+ob1
        nc.tensor.matmul(prz, wgo[:, 0:128], gl, start=True, stop=False, **kw)
        nc.tensor.matmul(pin, wgo[:, 128:192], gl, start=True, stop=False, **kw)

        # --- z stream: 8 x 1MiB f16 DMAs.  TensorReduce runs at 1x on
        # DVE but TensorTensor fp16 gets the 2x perf mode, so the spatial
        # sum is mostly a binary tree of fp16 adds on DVE (rows h=0..2),
        # with the h=3 row's four [128,256] chunks split between ACT
        # (Copy+accum_out) and Pool (STT+accum_out) so each engine stays
        # under the 2.9us DMA time.  The last DMA is split in half so the
        # final tree only trails the stream by ~1.4us.
        # Row d*2048 + h*512 + 4p + j -> batch b = 4d+h, channel 4p+j.
        hTc = hpool.tile([128, B_SH, J], f16)
        junk_a = hpool.tile([128, S], f16)
        z_r = z[:].rearrange("(d h p j) s -> d p h j s", h=H_PER, p=128, j=J)

        def tree_reduce(src_ap, n_rows, out_ap):
            # src_ap [128, n_rows, S] f16 -> out_ap [128, n_rows, 1]:
            # 4 halving TT stages then one 1x multi-axis reduce of 16.
            scA = work.tile([128, n_rows, S // 2], f16, tag=f"trA{n_rows}")
            scB = work.tile([128, n_rows, S // 4], f16, tag=f"trB{n_rows}")
            n = S // 2
            nc.vector.tensor_add(scA[:, :, 0:n], src_ap[:, :, 0:n],
                                 src_ap[:, :, n:2 * n])
            cur, oth = scA, scB
            while n > 16:
                h_n = n // 2
                nc.vector.tensor_add(oth[:, :, 0:h_n], cur[:, :, 0:h_n],
                                     cur[:, :, h_n:n])
                cur, oth = oth, cur
                n = h_n
            nc.vector.tensor_reduce(out=out_ap, in_=cur[:, :, 0:16],
                                    axis=AX.X, op=ALU.add)

        def act_chunks(zt, h, b, js):
            for j in js:
                nc.scalar.activation(
                    out=junk_a, in_=zt[:, h, j, :], func=AF.Copy,
                    accum_out=hTc[:, b, j:j + 1])

        for d in range(N_DMA - 1):
            zt = zpool.tile([128, H_PER, J, S], f16, tag="zt")
            nc.sync.dma_start(out=zt, in_=z_r[d])
            b = H_PER * d
            tree_reduce(
                zt[:].rearrange("p h j s -> p (h j) s")[:, 0:13, :], 13,
                hTc[:, b:b + 4, :].rearrange("p b j -> p (b j) ()")[:, 0:13, :])
            act_chunks(zt, 3, b + 3, range(1, 4))
        d = N_DMA - 1
        for half in range(2):
            zh = zpool.tile([128, 2, J, S], f16, tag="zh")
            nc.sync.dma_start(out=zh, in_=z_r[d][:, 2 * half:2 * half + 2])
            b = H_PER * d + 2 * half
            # DVE: row 0 fully + half of row 1; ACT: the other half
            tree_reduce(
                zh[:].rearrange("p h j s -> p (h j) s")[:, 0:6, :], 6,
                hTc[:, b:b + 2, :].rearrange("p b j -> p (b j) ()")[:, 0:6, :])
            act_chunks(zh, 1, b + 1, range(2, 4))
        # --- MLP weights queued after the z stream (needed later) ---
        w1 = consts.tile([128, 4, 512], f16)
        jw1_r = w1d[:].rearrange("(k p) m -> k p m", p=128)
        for k in range(0, 4, 2):
            nc.sync.dma_start(out=w1[:, k:k + 2, :], in_=jw1_r[k:k + 2])
        w2 = consts.tile([128, 4, 256], f16)
        nc.sync.dma_start(out=w2, in_=w2d[:].rearrange("(k p) m -> p k m", p=128))
        w34 = consts.tile([128, 320], f16)
        nc.sync.dma_start(out=w34, in_=w34d[:])

        # --- join MLP (transposed): hN_T = relu(W @ h_T + b) ---
        # bias+relu fused on Pool: (psum + bias) max 0 -> f16
        h1 = hpool.tile([128, 4, B_SH], f16)
        for m in range(4):
            pt = psum_mlp.tile([128, B_SH], f32, tag="mlp")
            for k in range(4):
                nc.tensor.matmul(pt, w1[:, k, m * 128:(m + 1) * 128],
                                 hTc[:, :, k],
                                 start=(k == 0), stop=(k == 3))
            nc.vector.tensor_scalar(
                out=h1[:, m, :], in0=pt, scalar1=wb[:, m:m + 1], scalar2=0.0,
                op0=ALU.add, op1=ALU.max)
        h2 = hpool.tile([128, 2, B_SH], f16)
        for m in range(2):
            pt = psum_mlp.tile([128, B_SH], f32, tag="mlp")
            for k in range(4):
                nc.tensor.matmul(pt, w2[:, k, m * 128:(m + 1) * 128], h1[:, k, :],
                                 start=(k == 0), stop=(k == 3))
            nc.vector.tensor_scalar(
                out=h2[:, m, :], in0=pt, scalar1=wb[:, 4 + m:5 + m], scalar2=0.0,
                op0=ALU.add, op1=ALU.max)
        h3 = hpool.tile([128, B_SH], f16)
        pt = psum_mlp.tile([128, B_SH], f32, tag="mlp")
        for k in range(2):
            nc.tensor.matmul(pt, w34[:, k * 128:(k + 1) * 128], h2[:, k, :],
                             start=(k == 0), stop=(k == 1))
        nc.vector.tensor_scalar(
            out=h3, in0=pt, scalar1=wb[:, 6:7], scalar2=0.0,
            op0=ALU.add, op1=ALU.max)
        pt = psum_mlp.tile([64, B_SH], f32, tag="mlp")
        nc.tensor.matmul(pt, w34[:, 256:320], h3, start=True, stop=True)
        nc.vector.tensor_scalar(
            out=hhg[0:64, :], in0=pt, scalar1=wb[0:64, 7:8], scalar2=0.0,
            op0=ALU.add, op1=ALU.max)

        # GRU hidden-path init matmuls (f32 operands, one-time).
        nc.tensor.matmul(prz, whh[:, 0:128], hhg, start=False, stop=False, **kw)
        nc.tensor.matmul(phn, whh[:, 128:192], hhg, start=True, stop=False, **kw)
        nc.tensor.matmul(pd1, ow1, hhg, start=True, stop=False, **kw)

        amr_junk = hpool.tile([64, 1], f32)
        # --- GRU: persistent psum accumulators, 8 unrolled steps.
        # DVE runs the elementwise chain (PSUM-capable); Pool takes the
        # off-chain hh update; ACT does sigmoid/tanh.  The sigmoid output
        # lands in PSUM (cheaper ACT access), reusing the idle MLP banks.
        x_prev = None
        for t in range(T):
            last = t == T - 1
            r_t = work.tile([64, B_SH], f32, tag="r_t")
            nc.scalar.activation(r_t, prz[0:64, :], AF.Sigmoid)
            z_t = work.tile([64, B_SH], f32, tag="z_t")
            nc.scalar.activation(z_t, prz[64:128, :], AF.Sigmoid)
            tmp = work.tile([64, B_SH], f32, tag="tmp")
            nc.vector.tensor_mul(tmp, r_t, phn)             # r * h_n
            ptm = psum_gru.tile([64, B_SH], f32, tag="ptm")
            nc.vector.tensor_add(ptm, tmp, pin)             # + i_n
            n_t = work.tile([64, B_SH], f32, tag="n_t")
            nc.scalar.activation(n_t, ptm, AF.Tanh)
            t1 = work.tile([64, B_SH], f32, tag="t1")
            nc.vector.tensor_sub(t1, hhg[0:64, :], n_t)     # hh - n
            # d = (1-z)*(hh-n) in one fused DVE op
            dlt = work.tile([64, B_SH], f16, tag="dlt")
            nc.vector.affine_mul_reduce(
                out=dlt, accum_out=amr_junk, in0=z_t, in1=t1,
                scale=-1.0, bias=1.0)

            # hh' = hh - d; pd1 first (it gates the output path); the
            # whhn updates are ready before the relu, so PE runs them
            # during the relu's sem latency.
            nc.tensor.matmul(pd1, ow1n, dlt,
                             start=False, stop=last, **kw)
            if not last:
                nc.tensor.matmul(prz, whhn[:, 0:128], dlt,
                                 start=False, stop=False, **kw)
                nc.tensor.matmul(phn, whhn[:, 128:192], dlt,
                                 start=False, stop=(t == T - 2), **kw)
                dlt32 = work.tile([64, B_SH], f32, tag="dlt32")
                nc.vector.affine_mul_reduce(
                    out=dlt32, accum_out=amr_junk, in0=z_t, in1=t1,
                    scale=-1.0, bias=1.0)
                nc.gpsimd.tensor_sub(hhg[0:64, :], hhg[0:64, :], dlt32)
            nc.vector.tensor_scalar_max(d1g[0:4, :], pd1, 0.0)  # d1(hh')
            if not last:
                # x-recurrence folded through d1g
                nc.tensor.matmul(prz, wixo[:, 0:128], d1g,
                                 start=False, stop=(t == T - 2), **kw)
                nc.tensor.matmul(pin, wixo[:, 128:192], d1g,
                                 start=False, stop=(t == T - 2), **kw)

            # x output (off the critical chain)
            pd3 = psum_gru.tile([3, B_SH], f32, tag="pd3")
            nc.tensor.matmul(pd3, ow23, d1g, start=True, stop=True)
            x_new = xpool.tile([3, B_SH], f32, tag="x")
            if x_prev is None:
                nc.vector.tensor_copy(x_new, pd3)
            else:
                nc.vector.tensor_add(x_new, x_prev, pd3)
            nc.sync.dma_start(out=out_d[3 * t:3 * t + 3, :], in_=x_new)
            x_prev = x_new

    nc.compile()
    return nc


def _get_program():
    if "nc" not in _CACHE:
        _CACHE["nc"] = _build_program()
    return _CACHE["nc"]


def make_in_maps(**inputs) -> list[dict]:
    """Host-side packing + data-parallel sharding -> one in_map per core."""
    f = lambda a: np.ascontiguousarray(np.asarray(a, dtype=np.float32))
    z = f(inputs["z"]).reshape(B, C, S)
    gp = f(inputs["goal_point"])
    gps = f(inputs["goal_point_speed"])
    W_ih, W_hh = f(inputs["W_ih"]), f(inputs["W_hh"])
    b_ih, b_hh = f(inputs["b_ih"]), f(inputs["b_hh"])
    oW1, ob1 = f(inputs["oW1"]), f(inputs["ob1"])
    oW2, ob2 = f(inputs["oW2"]), f(inputs["ob2"])
    oW3, ob3 = f(inputs["oW3"]), f(inputs["ob3"])

    # layer-1 weight: fold the 1/S mean scale and the z-layout channel
    # permutation (chunk j, partition p <-> channel 4p+j).
    jw1t = f(inputs["jW1"]).T * np.float32(1.0 / S)
    perm = (4 * np.arange(128)[None, :] + np.arange(4)[:, None]).reshape(-1)
    w1 = np.ascontiguousarray(jw1t[perm]).astype(np.float16)
    w2 = np.ascontiguousarray(f(inputs["jW2"]).T).astype(np.float16)
    jw3t = f(inputs["jW3"]).T.astype(np.float16)                 # [256, 128]
    jw4t = f(inputs["jW4"]).T.astype(np.float16)                 # [128, 64]
    w34 = np.zeros((128, 320), np.float16)
    w34[:, 0:128] = jw3t[0:128]
    w34[:, 128:256] = jw3t[128:256]
    w34[:, 256:320] = jw4t

    # bias pack [128, 8]: jb1 (4 cols), jb2 (2), jb3 (1), jb4 (1, rows 0:64)
    wbias = np.zeros((128, 8), np.float32)
    wbias[:, 0:4] = f(inputs["jb1"]).reshape(4, 128).T
    wbias[:, 4:6] = f(inputs["jb2"]).reshape(2, 128).T
    wbias[:, 6] = f(inputs["jb3"])
    wbias[0:64, 7] = f(inputs["jb4"])

    brow = np.concatenate([b_ih[0:128] + b_hh[0:128], b_ih[128:192]])
    wgobt = np.concatenate([W_ih[:, 3:6].T, brow[None, :]])  # [4, 192]
    brow2 = np.concatenate([np.zeros(128, np.float32), b_hh[128:192]])
    whhbt = np.concatenate([W_hh.T, brow2[None, :]])         # [65, 192]
    ow1bt = np.concatenate([oW1.T, ob1[None, :]])            # [65, 4]

    w23 = oW2.T @ oW3.T                                      # [4, 3]
    b23 = ob2 @ oW3.T + ob3                                  # [3]
    wp16 = np.zeros((64, P16_COLS), np.float16)
    wp16[0:64, 0:192] = -W_hh.T
    wp16[0:4, 192:384] = w23 @ W_ih[:, 0:3].T
    wp16[32, 192:384] = W_ih[:, 0:3] @ b23
    wp16[0:64, 384:388] = -oW1.T
    wp16[0:4, 388:391] = w23
    wp16[32, 388:391] = b23

    goalT = np.stack([gp[:, 0, 3], gp[:, 1, 3], gps])        # [3, 256]

    z16 = z.astype(np.float16)

    in_maps = []
    for i in range(N_CORES):
        sl = slice(i * B_SH, (i + 1) * B_SH)
        wp32 = np.zeros((65, P32_COLS), np.float32)
        wp32[0:65, 0:192] = whhbt
        wp32[0:4, 192:384] = wgobt
        wp32[0:3, 384:384 + B_SH] = goalT[:, sl]
        wp32[3, 384:384 + B_SH] = 1.0
        wp32[0:65, 416:420] = ow1bt
        in_maps.append(dict(
            z=np.ascontiguousarray(z16[sl].reshape(ROWS, S)),
            w1=w1, w2=w2, w34=w34, wbias=wbias,
            wp32=wp32, wp16=wp16,
        ))
    return in_maps


def unshard_out(results: list[dict]) -> np.ndarray:
    # per-core out [24, 32]: row 3t+c, col b  ->  [32, 8, 3]
    parts = [r["out"].reshape(T, 3, B_SH).transpose(2, 0, 1) for r in results]
    return np.ascontiguousarray(np.concatenate(parts, axis=0), dtype=np.float32)


def kernel(**inputs) -> np.ndarray:
    from concourse.bass_utils import run_bass_kernel_spmd

    nc = _get_program()
    in_maps = make_in_maps(**inputs)
    res = run_bass_kernel_spmd(nc, in_maps, core_ids=list(range(N_CORES)))
    return unshard_out(res.results)
